# revision 16
# baseline (speedup 1.0000x reference)
"""Trainium2 Bass kernel: spiking multi-head attention (nn_MultiHeadedAttention).

Reference semantics (B=4, T=2048, DIN=100, D=512, h=8 heads, dk=64):
    q = spike(query @ Wq + bq)   (spike = (x >= 1.0) -> {0,1})
    k = spike(key @ Wk + bk);  v = spike(value @ Wv + bv)
    attn = (q @ k^T) * scale, causally masked (keep k<=q), NO softmax
    x = spike(attn @ v)
    x = x.transpose(0,1,3,2).reshape(B,T,h*dk)    # scrambled reshape
    y = spike(x @ Wo + bo)

Key facts exploited:
  * No softmax -> causal attention is LINEAR attention:
        O_t = q_t . M_t  +  intra-block tril(Q K^T) V,   M = sum_j k_j v_j^T
    so only the 16 diagonal 128x128 S-tiles per head are materialized.
  * The scrambled reshape maps output rows [256*h, 256*(h+1)) to exactly one
    head h, so head-parallel sharding needs NO cross-core communication.
  * float32r matmuls stream 1 col/cycle for N>=256 (4x fp32, measured 227ns
    vs 860ns at N=512) with fp32-range 12-bit-mantissa operands; the host
    pre-rounds all real-valued operands to the f32r grid so every projection
    runs at bf16 speed.  End-to-end spike-flip error from the rounding is
    ~1.3e-2 rel (gate 2e-2), verified against the reference on CPU.
  * Spiked tensors are {0,1} and S / O / M are integers, all exact in fp16;
    the attention core uses fp16 operands AND fp16 PSUM tiles (integer
    values -> exact; halves DVE PSUM-read cost and PSUM bank usage).
  * O_inter is ONE matmul per head pair (lhsT = full 128-row qs tile times a
    block-diag-masked M snapshot), halving its LDWEIGHTS cost.
  * Final projection: xs {0,1} fp16 x Wo fp16 (adds ~51 spike flips), y
    spike emitted as uint8 (host upcasts) cutting output DMA 4x.

Sharding: core c -> batch b=c//2, head-group hg=c%2 (4 heads per core).

Hardware notes encoded below:
  * K=64 matmuls whose lhsT sits at partition base 0 vs base 64 execute
    concurrently in disjoint PE row groups; concurrent writes to one PSUM
    bank hang the device -> parity-0/1 S tiles live in different banks
    (PSUM pool slots are bank-padded, one slot per tag).
  * PSUM budget is exactly 8 banks: pp(2, fp32 projections+final) ps(2,
    fp16 S parities) po(2, fp16 O pairs) pm(1, fp16 M state) tp(1, fp16
    transposes).
  * Engine balance: DVE does the tensor_tensor masks + y/kn; ACT+GPSIMD
    run the two-op spike chains (GPSIMD cannot read PSUM, so it always
    takes the SBUF-side second op).
  * DMA-issue instructions are expensive on the issuing engine; all weights
    ride one packed f32r tensor and k/v loads are paced with gate-copies so
    early-needed pieces get full ring bandwidth.
"""

import os
import numpy as np

B, T, DIN, D = 4, 2048, 100, 512
H, DK = 8, 64
NCORES = 8
HPC = 4          # heads per core
DH = HPC * DK    # 256 projected features per core
P = 128
NT = T // P      # 16 t-blocks
KC = D // P      # 4 contraction chunks of the D=512 dim
NPIECE = 4       # pieces along T (512 each)

# packed weights ride in two tensors: wpk (f32r: everything consumed by
# f32r matmuls -- the compiler re-rounds f32r DRAM data, so bit-packed f16
# payloads must NOT live there) and wpx (plain f32: Wo f16 pairs + masks).
OFF_WK = 0                      # 4 chunks x 256 = 1024 f32r cols
OFF_WV = 1024                   # 1024 cols
OFF_WQ = 2048                   # 256 cols
OFF_BIAS = 2304                 # rows 0/1 = bk/bv (f32r)
WPACK_W = 2560                  # f32r tensor width
XOFF_WO = 0                     # Wo fp16 pairs: 4 chunks x 512 f16 = 1024 f32 cols
XOFF_MSK = 1024                 # triu mask f32: 256 cols
XOFF_MDG = 1280                 # block-diag mask f32: 128 cols
XOFF_BO = 1408                  # row 0 = bo (fp16 pairs, 256 f32 cols)
WPACKX_W = 1664                 # f32 tensor width

_prog_cache: dict = {}
last_exec_time_ns = None


def _f32r_round(x: np.ndarray) -> np.ndarray:
    """Round fp32 to the f32r grid (11 explicit mantissa bits, RN)."""
    u = np.ascontiguousarray(x, np.float32).view(np.uint32)
    r = (u + np.uint32(0x7FF) + ((u >> np.uint32(12)) & np.uint32(1))) & np.uint32(
        0xFFFFF000
    )
    return r.view(np.float32)


def _build(scale: float, has_bk: bool, has_bv: bool, has_bo: bool):
    from contextlib import ExitStack

    import concourse.bass as bass
    import concourse.tile as tile
    import concourse.mybir as mybir
    from concourse import bacc
    from concourse.bass import ts
    from concourse import masks

    f32 = mybir.dt.float32
    f32r = mybir.dt.float32r
    f16 = mybir.dt.float16
    u8 = mybir.dt.uint8
    ALU = mybir.AluOpType
    AF = mybir.ActivationFunctionType
    inv_scale = 1.0 / scale

    nc = bacc.Bacc(
        "TRN2", target_bir_lowering=False, debug=False, num_devices=NCORES
    )

    # DRAM I/O (host pre-transposes and pre-rounds to the f32r grid; qT
    # carries an extra all-ones row so bq rides in Wq's last row).
    qT = nc.dram_tensor("qT", [DIN + 1, T], f32r, kind="ExternalInput").ap()
    kT = nc.dram_tensor("kT", [D, T], f32r, kind="ExternalInput").ap()
    vT = nc.dram_tensor("vT", [D, T], f32r, kind="ExternalInput").ap()
    wpk = nc.dram_tensor("wpk", [P, WPACK_W], f32r, kind="ExternalInput").ap()
    wpx = nc.dram_tensor("wpx", [P, WPACKX_W], f32, kind="ExternalInput").ap()
    y = nc.dram_tensor("y", [HPC * 256, D], u8, kind="ExternalOutput").ap()

    with tile.TileContext(nc) as tc, ExitStack() as ctx:
        pool = lambda name, bufs, space="SBUF": ctx.enter_context(
            tc.tile_pool(name=name, bufs=bufs, space=space)
        )
        persist = pool("persist", 1)      # distinct tags -> own slots
        s_pool = pool("s_pool", 3)        # masked S tiles (fp16)
        t_pool = pool("t_pool", 4)        # spike-chain temporaries
        m_pool = pool("m_pool", 2)        # M snapshots (block-diag masked)
        y_pool = pool("y_pool", 3)        # output staging
        pp = pool("pp", 2, "PSUM")        # projections + final (fp32)
        ps = pool("ps", 1, "PSUM")        # S tiles (fp16, 2 parity tags)
        po = pool("po", 2, "PSUM")        # O accumulators (fp16, pair tiles)
        pm = pool("pm", 1, "PSUM")        # persistent M state (fp16)
        pt_ps = pool("pt_ps", 1, "PSUM")  # transpose staging (fp16)

        def ptile(shape, dtype, *, name):
            return persist.tile(shape, dtype, name=name, tag=name)

        # ---- SBUF allocations -----------------------------------------
        qt_sb = ptile([P, T], f32r, name="qt_sb")
        # per-piece k/v tiles: cols = 512*c + j (all 4 contraction chunks of
        # one 512-wide t-piece).  One DMA issue per tile keeps the
        # dependency of each piece's compute narrow.
        PW = T // NPIECE
        kt_p = [ptile([P, KC * PW], f32r, name=f"kt_p{pc}") for pc in range(NPIECE)]
        vt_p = [ptile([P, KC * PW], f32r, name=f"vt_p{pc}") for pc in range(NPIECE)]
        wp_sb = ptile([P, WPACK_W], f32r, name="wp_sb")
        wx_sb = ptile([P, WPACKX_W], f32, name="wx_sb")
        wk_sb = [wp_sb[:, OFF_WK + 256 * c :][:, 0:DH] for c in range(KC)]
        wv_sb = [wp_sb[:, OFF_WV + 256 * c :][:, 0:DH] for c in range(KC)]
        wq_sb = wp_sb[:, OFF_WQ : OFF_WQ + DH]
        wo_sb = [
            wx_sb[:, XOFF_WO + 256 * c : XOFF_WO + 256 * (c + 1)].bitcast(f16)
            for c in range(KC)
        ]
        msk_sb = wx_sb[:, XOFF_MSK : XOFF_MSK + 2 * P]               # [128,256]
        mdg_sb = wx_sb[:, XOFF_MDG : XOFF_MDG + P]                   # [128,128]
        bias_sb = wp_sb[:, OFF_BIAS:WPACK_W]
        bo_sb = wx_sb[:, XOFF_BO : XOFF_BO + 2 * P].bitcast(f16)     # [128,512]
        ones_sb = ptile([1, D], f32r, name="ones_sb")
        idt_sb = ptile([P, P], f16, name="idt_sb")
        idt_ones = ptile([1, P], f16, name="idt_ones")
        if has_bo:
            nc.vector.memset(idt_ones[:, :], 1.0)
        # qs/ks: spiked projections, d-major [dk, T]; tile i holds heads
        # 2i (parts 0:64) and 2i+1 (parts 64:128).
        qs = [ptile([P, T], f16, name=f"qs{i}") for i in range(2)]
        ks = [ptile([P, T], f16, name=f"ks{i}") for i in range(2)]
        # vkn: t-major spiked v for all 4 heads (cols 256t + 64*hl), fp16.
        vkn = ptile([P, DH * NT], f16, name="vkn")
        # kn: t-major spiked k, pair-major: cols 256t + 128*pair + 64*(hl%2)
        kn = ptile([P, DH * NT], f16, name="kn")
        # xs: spiked attention output, xs[p, 256*tt + 128*i + 64*sub + d]
        # (head h = 2i+sub): both the per-block spike write and the final
        # projection lhsT are fully contiguous [128,128] views.
        xs = ptile([P, 1024 * HPC], f16, name="xs")

        # ---- loads ----------------------------------------------------
        # The DMA ring fair-shares bandwidth across in-flight transfers;
        # tiny gate-copies (read prev dest, write next dest) order them.
        # Two parallel HWDGE FIFO rings: sync (qSPDynamicHW) carries wpk,
        # qt, vt and wpx; scalar (qActDynamicHW) carries kt.  Each piece's
        # transfers are issued JUST BEFORE the compute that consumes the
        # previous piece, so completion-semaphore waits never cover
        # later-issued transfers.
        ktv = kT.rearrange("(c p) t -> p c t", c=KC)
        vtv = vT.rearrange("(c p) t -> p c t", c=KC)

        def load_piece(pc):
            if pc > 0:
                gate(kt_p[pc - 1][0:1, 0:1], kt_p[pc][0:1, 0:1])
            nc.scalar.dma_start(
                out=kt_p[pc].rearrange("p (c j) -> p c j", c=KC),
                in_=ktv[:, :, ts(pc, PW)],
            )
            if pc > 0:
                gate(
                    (wx_sb[0:1, 0:1].bitcast(f32r) if pc == 1
                     else vt_p[pc - 1][0:1, 0:1]),
                    vt_p[pc][0:1, 0:1],
                )
            nc.sync.dma_start(
                out=vt_p[pc].rearrange("p (c j) -> p c j", c=KC),
                in_=vtv[:, :, ts(pc, PW)],
            )

        # The DMA engines fair-share across ALL in-flight transfers, so
        # without ordering every transfer completes near the END of the
        # load phase.  Tiny gate-copies (read the previous transfer's
        # destination, write the next one's) serialize each ring so early
        # pieces finish early and compute can start at ~7us.
        def gate(prev_ap, next_ap):
            nc.vector.tensor_copy(next_ap, prev_ap)

        nc.sync.dma_start(out=wp_sb[:, :], in_=wpk[:, :])
        gate(wp_sb[0:1, 0:1], qt_sb[0:1, 0:1])
        nc.sync.dma_start(out=qt_sb[: DIN + 1, :], in_=qT[:, :])
        gate(qt_sb[0:1, 0:1], vt_p[0][0:1, 0:1])
        load_piece(0)
        gate(vt_p[0][0:1, 0:1].bitcast(f32), wx_sb[0:1, 0:1])
        nc.sync.dma_start(out=wx_sb[:, :], in_=wpx[:, :])
        nc.vector.memset(ones_sb[:, :].bitcast(f32), 1.0)
        masks.make_identity(nc, idt_sb[:, :])

        BIG = float(2 ** 26)

        def spike_chain(out_ap, in_ap, nm):
            """spike(x) via two exact Relu ops on the ACT engine."""
            tmp = t_pool.tile([P, in_ap.free_size()],
                              f32, name=f"tmp_{nm}", tag=f"tmp_{nm}")
            nc.scalar.activation(tmp[:, :], in_ap, AF.Relu, bias=1.0, scale=-1.0)
            nc.scalar.activation(out_ap, tmp[:, :], AF.Relu, bias=1.0, scale=-BIG)

        # ---- per-piece projections ------------------------------------
        def q_piece(pc):
            for half in range(2):
                pt = pp.tile([P, 512], f32, name="pt", tag="pt")
                nc.tensor.matmul(
                    pt[:, :],
                    lhsT=wq_sb[: DIN + 1, ts(half, P)],
                    rhs=qt_sb[: DIN + 1, ts(pc, 512)],
                    start=True,
                    stop=True,
                )
                spike_chain(qs[half][:, ts(pc, 512)], pt[:, :], "q")

        def ks_chunk(ch):
            for half in range(2):
                pt = pp.tile([P, 512], f32, name="pt", tag="pt")
                for c in range(KC):
                    nc.tensor.matmul(
                        pt[:, :],
                        lhsT=wk_sb[c][:, ts(half, P)],
                        rhs=kt_p[ch][:, ts(c, PW)],
                        start=(c == 0),
                        stop=(c == KC - 1) and not has_bk,
                    )
                if has_bk:
                    nc.tensor.matmul(
                        pt[:, :],
                        lhsT=bias_sb[0:1, ts(half, P)],
                        rhs=ones_sb[0:1, 0:512],
                        start=False,
                        stop=True,
                    )
                spike_chain(ks[half][:, ts(ch, 512)], pt[:, :], "k")
            # t-major spiked K via PE transpose; both pair tiles land in one
            # PSUM tile so a single DVE copy moves them (kn block is
            # contiguous in the pair-major layout the M-update wants).
            for tt in range(4 * ch, 4 * ch + 4):
                tp = pt_ps.tile([P, 2 * P], f16, name="tp", tag="tp")
                for pr in range(2):
                    nc.tensor.transpose(
                        tp[:, ts(pr, P)], ks[pr][:, ts(tt, P)], idt_sb[:, :]
                    )
                nc.vector.tensor_copy(kn[:, ts(tt, DH)], tp[:, :])

        def vkn_block(tt):
            pt = pp.tile([P, 512], f32, name="pt", tag="pt")
            for c in range(KC):
                nc.tensor.matmul(
                    pt[:, 0:DH],
                    lhsT=vt_p[tt // 4][:, PW * c + P * (tt % 4) :][:, 0:P],
                    rhs=wv_sb[c][:, :],
                    start=(c == 0),
                    stop=(c == KC - 1) and not has_bv,
                )
            if has_bv:
                nc.tensor.matmul(
                    pt[:, 0:DH],
                    lhsT=ones_sb[0:1, 0:P],
                    rhs=bias_sb[1:2, 0:DH],
                    start=False,
                    stop=True,
                )
            spike_chain(vkn[:, ts(tt, DH)], pt[:, 0:DH], "v")

        # ---- attention ------------------------------------------------
        pm_t = pm.tile([P, DH], f32, name="pm_t")

        def attn_block(tt):
            if tt > 0:
                # snapshot M_(<tt), block-diag masked so the merged
                # per-pair O_inter matmul sees zero cross-head terms.
                m_sb = [
                    m_pool.tile([P, P], f16, name=f"m_sb{i}", tag=f"m_sb{i}")
                    for i in range(2)
                ]
                for i in range(2):
                    nc.vector.tensor_tensor(
                        m_sb[i][:, :], pm_t[:, ts(i, P)], mdg_sb[:, :],
                        op=ALU.mult,
                    )
            else:
                m_sb = None
            s_ps = [
                ps.tile([P, DH], f32, name=f"s_ps{par}", tag=f"s_ps{par}")
                for par in range(2)
            ]
            for hl in range(HPC):
                par, idx = hl % 2, hl // 2
                rows = slice(64 * par, 64 * par + 64)
                nc.tensor.matmul(
                    s_ps[par][:, ts(idx, P)],
                    lhsT=ks[idx][rows, ts(tt, P)],
                    rhs=qs[idx][rows, ts(tt, P)],
                    start=True,
                    stop=True,
                )
            s_sb = [
                s_pool.tile([P, DH], f16, name=f"s_sb{par}", tag=f"s_sb{par}")
                for par in range(2)
            ]
            for par in range(2):
                nc.vector.tensor_tensor(
                    s_sb[par][:, :], s_ps[par][:, :], msk_sb[:, :], op=ALU.mult
                )
            # o_ps[i]: [tq, dv of heads 2i (cols 0:64), 2i+1 (64:128)]
            # The two intra matmuls share ONE start: start=True clears the
            # has_written bits of the whole PSUM zero region, so a second
            # start would make the full-width O_inter OVERWRITE (not
            # accumulate onto) the first head's columns.
            o_ps = [po.tile([P, P], f32, name="o_ps") for _ in range(2)]
            for i in range(2):
                for par in range(2):
                    hl = 2 * i + par
                    nc.tensor.matmul(
                        o_ps[i][:, ts(par, 64)],
                        lhsT=s_sb[par][:, ts(i, P)],
                        rhs=vkn[:, DH * tt + 64 * hl :][:, 0:64],
                        start=(par == 0),
                        stop=(tt == 0),
                        skip_group_check=True,
                    )
            if tt > 0:
                for i in range(2):
                    nc.tensor.matmul(
                        o_ps[i][:, :],
                        lhsT=qs[i][:, ts(tt, P)],
                        rhs=m_sb[i][:, :],
                        start=False,
                        stop=True,
                        skip_group_check=True,
                    )
            # M += K_pair^T V_pair: one K=128,N=128 matmul per head pair;
            # only the diagonal 64x64 blocks are meaningful (snapshot is
            # masked).  stop=True closes the group so the snapshot read is
            # legal; on HW stop is a no-op and start=False keeps summing.
            for pr in range(2):
                nc.tensor.matmul(
                    pm_t[:, ts(pr, P)],
                    lhsT=kn[:, DH * tt + P * pr :][:, 0:P],
                    rhs=vkn[:, DH * tt + P * pr :][:, 0:P],
                    start=(tt == 0 and pr == 0),
                    stop=(pr == 1),
                    skip_group_check=True,
                )
            # x = spike(scale * O) = (O >= 1/scale): one DVE op per pair.
            for i in range(2):
                nc.vector.tensor_scalar(
                    xs[:, DH * tt + P * i :][:, 0:P],
                    o_ps[i][:, :],
                    inv_scale,
                    None,
                    ALU.is_ge,
                )

        # ---- final projection (fp16: xs {0,1} x Wo fp16) --------------
        # Output rows r with r%4 == m contract only over attention piece m
        # (X[r, f] = x_att[t=512*(r%4)+f, d=r//4]).  With the xs layout the
        # lhsT for (piece m, chunk cc, pair j) is the contiguous block
        # xs[:, 256*(4m+cc) + 128j :][:128].

        def final_piece(m):
            for j in range(2):  # head pair: heads 2j, 2j+1
                yp = pp.tile([P, 512], f32, name="pt", tag="pt")
                for cc in range(KC):
                    nc.tensor.matmul(
                        yp[:, :],
                        lhsT=xs[:, DH * (4 * m + cc) + P * j :][:, 0:P],
                        rhs=wo_sb[cc][:, :],
                        start=(cc == 0),
                        stop=(cc == KC - 1) and not has_bo,
                    )
                if has_bo:
                    nc.tensor.matmul(
                        yp[:, :],
                        lhsT=idt_ones[0:1, 0:P],
                        rhs=bo_sb[0:1, 0:512],
                        start=False,
                        stop=True,
                    )
                y_sb = y_pool.tile([P, D], u8, name="y_sb")
                nc.vector.tensor_scalar(
                    y_sb[:, :], yp[:, :], 1.0, None, ALU.is_ge
                )
                for sub in range(2):
                    h = 2 * j + sub
                    nc.gpsimd.dma_start(
                        out=y[256 * h + m : 256 * (h + 1) : 4, :],
                        in_=y_sb[64 * sub : 64 * sub + 64, :],
                    )

        # ---- schedule -------------------------------------------------
        def proj_piece(pc):
            q_piece(pc)
            ks_chunk(pc)
            for tt in range(4 * pc, 4 * pc + 4):
                vkn_block(tt)

        load_piece(1)
        proj_piece(0)
        load_piece(2)
        proj_piece(1)
        for pc in range(4):
            if pc + 2 < 4:
                if pc + 3 < 4:
                    load_piece(pc + 3)
                proj_piece(pc + 2)
            for tt in range(4 * pc, 4 * pc + 4):
                attn_block(tt)
            final_piece(pc)

    nc.compile()
    return nc


def _get_prog(scale, has_bk, has_bv, has_bo):
    key = (scale, has_bk, has_bv, has_bo)
    if key not in _prog_cache:
        _prog_cache[key] = _build(scale, has_bk, has_bv, has_bo)
    return _prog_cache[key]


def _pack_weights(Wq, bq, Wk, bk, Wv, bv, Wo, bo, cs):
    wpk = np.zeros((P, WPACK_W), np.float32)
    wpx = np.zeros((P, WPACKX_W), np.float32)
    for c in range(KC):
        wpk[:, OFF_WK + 256 * c : OFF_WK + 256 * (c + 1)] = _f32r_round(
            Wk[128 * c : 128 * (c + 1), cs]
        )
        wpk[:, OFF_WV + 256 * c : OFF_WV + 256 * (c + 1)] = _f32r_round(
            Wv[128 * c : 128 * (c + 1), cs]
        )
        # Wo fp16 pairs packed into f32 words
        wo16 = np.ascontiguousarray(
            Wo[128 * c : 128 * (c + 1), :].astype(np.float16)
        )
        wpx[:, XOFF_WO + 256 * c : XOFF_WO + 256 * (c + 1)] = wo16.view(
            np.float32
        )
    wq = np.zeros((P, DH), np.float32)
    wq[:DIN] = Wq[:, cs]
    wq[DIN] = bq[cs]
    wpk[:, OFF_WQ : OFF_WQ + DH] = _f32r_round(wq)
    wpx[:, XOFF_MSK : XOFF_MSK + 2 * P] = np.tile(
        np.triu(np.ones((P, P), np.float32)), (1, 2)
    )
    wpx[:, XOFF_MDG : XOFF_MDG + P] = np.kron(
        np.eye(2, dtype=np.float32), np.ones((64, 64), np.float32)
    )
    wpk[0, OFF_BIAS : OFF_BIAS + DH] = _f32r_round(
        np.ascontiguousarray(bk[cs], np.float32)
    )
    wpk[1, OFF_BIAS : OFF_BIAS + DH] = _f32r_round(
        np.ascontiguousarray(bv[cs], np.float32)
    )
    bo16 = np.ascontiguousarray(bo.astype(np.float16))
    wpx[0, XOFF_BO : XOFF_BO + D // 2] = bo16.view(np.float32)
    return wpk, wpx


def kernel(**inputs) -> np.ndarray:
    global last_exec_time_ns
    from concourse.bass_utils import run_bass_kernel_spmd

    g = lambda n: np.asarray(inputs[n], dtype=np.float32)
    query, key, value = g("query"), g("key"), g("value")
    Wq, bq, Wk, bk = g("Wq"), g("bq"), g("Wk"), g("bk")
    Wv, bv, Wo, bo = g("Wv"), g("bv"), g("Wo"), g("bo")
    scale = float(np.asarray(inputs["scale"], dtype=np.float32).reshape(-1)[0])

    has_bk, has_bv, has_bo = (bool(np.any(x)) for x in (bk, bv, bo))
    prog = _get_prog(scale, has_bk, has_bv, has_bo)

    # pre-round the shared per-batch data once
    qTr, kTr, vTr = [None] * B, [None] * B, [None] * B
    for b in range(B):
        qTa = np.empty((DIN + 1, T), np.float32)
        qTa[:DIN] = _f32r_round(np.ascontiguousarray(query[b].T))
        qTa[DIN] = 1.0
        qTr[b] = qTa
        kTr[b] = _f32r_round(np.ascontiguousarray(key[b].T))
        vTr[b] = _f32r_round(np.ascontiguousarray(value[b].T))

    in_maps = []
    for c in range(NCORES):
        b, hg = divmod(c, 2)
        cs = slice(DH * hg, DH * (hg + 1))
        wpk_c, wpx_c = _pack_weights(Wq, bq, Wk, bk, Wv, bv, Wo, bo, cs)
        in_maps.append(
            {
                "qT": qTr[b],
                "kT": kTr[b],
                "vT": vTr[b],
                "wpk": wpk_c,
                "wpx": wpx_c,
            }
        )

    trace = os.environ.get("BASS_TRACE", "") not in ("", "0")
    res = run_bass_kernel_spmd(
        prog, in_maps, core_ids=list(range(NCORES)), trace=trace
    )
    last_exec_time_ns = res.exec_time_ns
    if res.exec_time_ns is not None:
        print(f"HW exec time: {res.exec_time_ns} ns")

    out = np.empty((B, T, D), np.float32)
    for c in range(NCORES):
        b, hg = divmod(c, 2)
        out[b, 1024 * hg : 1024 * (hg + 1)] = res.results[c]["y"].astype(
            np.float32
        )
    return out


# revision 17
# speedup vs baseline: 1.0318x; 1.0318x over previous
"""Trainium2 Bass kernel: spiking multi-head attention (nn_MultiHeadedAttention).

Reference semantics (B=4, T=2048, DIN=100, D=512, h=8 heads, dk=64):
    q = spike(query @ Wq + bq)   (spike = (x >= 1.0) -> {0,1})
    k = spike(key @ Wk + bk);  v = spike(value @ Wv + bv)
    attn = (q @ k^T) * scale, causally masked (keep k<=q), NO softmax
    x = spike(attn @ v)
    x = x.transpose(0,1,3,2).reshape(B,T,h*dk)    # scrambled reshape
    y = spike(x @ Wo + bo)

Key facts exploited:
  * No softmax -> causal attention is LINEAR attention:
        O_t = q_t . M_t  +  intra-block tril(Q K^T) V,   M = sum_j k_j v_j^T
    so only the 16 diagonal 128x128 S-tiles per head are materialized.
  * The scrambled reshape maps output rows [256*h, 256*(h+1)) to exactly one
    head h, so head-parallel sharding needs NO cross-core communication.
  * float32r matmuls stream 1 col/cycle for N>=256 (4x fp32, measured 227ns
    vs 860ns at N=512) with fp32-range 12-bit-mantissa operands; the host
    pre-rounds all real-valued operands to the f32r grid so every projection
    runs at bf16 speed.  End-to-end spike-flip error from the rounding is
    ~1.3e-2 rel (gate 2e-2), verified against the reference on CPU.
  * Spiked tensors are {0,1} and S / O / M are integers, all exact in fp16;
    the attention core uses fp16 operands AND fp16 PSUM tiles (integer
    values -> exact; halves DVE PSUM-read cost and PSUM bank usage).
  * O_inter is ONE matmul per head pair (lhsT = full 128-row qs tile times a
    block-diag-masked M snapshot), halving its LDWEIGHTS cost.
  * Final projection: xs {0,1} fp16 x Wo fp16 (adds ~51 spike flips), y
    spike emitted as uint8 (host upcasts) cutting output DMA 4x.

Sharding: core c -> batch b=c//2, head-group hg=c%2 (4 heads per core).

Hardware notes encoded below:
  * K=64 matmuls whose lhsT sits at partition base 0 vs base 64 execute
    concurrently in disjoint PE row groups; concurrent writes to one PSUM
    bank hang the device -> parity-0/1 S tiles live in different banks
    (PSUM pool slots are bank-padded, one slot per tag).
  * PSUM budget is exactly 8 banks: pp(2, fp32 projections+final) ps(2,
    fp16 S parities) po(2, fp16 O pairs) pm(1, fp16 M state) tp(1, fp16
    transposes).
  * Engine balance: DVE does the tensor_tensor masks + y/kn; ACT+GPSIMD
    run the two-op spike chains (GPSIMD cannot read PSUM, so it always
    takes the SBUF-side second op).
  * DMA-issue instructions are expensive on the issuing engine; all weights
    ride one packed f32r tensor and k/v loads are paced with gate-copies so
    early-needed pieces get full ring bandwidth.
"""

import os
import numpy as np

B, T, DIN, D = 4, 2048, 100, 512
H, DK = 8, 64
NCORES = 8
HPC = 4          # heads per core
DH = HPC * DK    # 256 projected features per core
P = 128
NT = T // P      # 16 t-blocks
KC = D // P      # 4 contraction chunks of the D=512 dim
NPIECE = 4       # pieces along T (512 each)

# packed weights ride in two tensors: wpk (f32r: everything consumed by
# f32r matmuls -- the compiler re-rounds f32r DRAM data, so bit-packed f16
# payloads must NOT live there) and wpx (plain f32: Wo f16 pairs + masks).
OFF_WK = 0                      # 4 chunks x 256 = 1024 f32r cols
OFF_WV = 1024                   # 1024 cols
OFF_WQ = 2048                   # 256 cols
OFF_BIAS = 2304                 # rows 0/1 = bk/bv (f32r)
WPACK_W = 2560                  # f32r tensor width
XOFF_WO = 0                     # Wo fp16 pairs: 4 chunks x 512 f16 = 1024 f32 cols
XOFF_MSK = 1024                 # triu mask f32: 256 cols
XOFF_MDG = 1280                 # block-diag mask f32: 128 cols
XOFF_BO = 1408                  # row 0 = bo (fp16 pairs, 256 f32 cols)
WPACKX_W = 1664                 # f32 tensor width

_prog_cache: dict = {}
last_exec_time_ns = None


def _f32r_round(x: np.ndarray) -> np.ndarray:
    """Round fp32 to the f32r grid (11 explicit mantissa bits, RN)."""
    u = np.ascontiguousarray(x, np.float32).view(np.uint32)
    r = (u + np.uint32(0x7FF) + ((u >> np.uint32(12)) & np.uint32(1))) & np.uint32(
        0xFFFFF000
    )
    return r.view(np.float32)


def _build(scale: float, has_bk: bool, has_bv: bool, has_bo: bool):
    from contextlib import ExitStack

    import concourse.bass as bass
    import concourse.tile as tile
    import concourse.mybir as mybir
    from concourse import bacc
    from concourse.bass import ts
    from concourse import masks

    f32 = mybir.dt.float32
    f32r = mybir.dt.float32r
    f16 = mybir.dt.float16
    u8 = mybir.dt.uint8
    ALU = mybir.AluOpType
    AF = mybir.ActivationFunctionType
    inv_scale = 1.0 / scale

    nc = bacc.Bacc(
        "TRN2", target_bir_lowering=False, debug=False, num_devices=NCORES
    )

    # DRAM I/O (host pre-transposes and pre-rounds to the f32r grid; qT
    # carries an extra all-ones row so bq rides in Wq's last row).
    qT = nc.dram_tensor("qT", [DIN + 1, T], f32r, kind="ExternalInput").ap()
    kT = nc.dram_tensor("kT", [D, T], f32r, kind="ExternalInput").ap()
    vT = nc.dram_tensor("vT", [D, T], f32r, kind="ExternalInput").ap()
    wpk = nc.dram_tensor("wpk", [P, WPACK_W], f32r, kind="ExternalInput").ap()
    wpx = nc.dram_tensor("wpx", [P, WPACKX_W], f32, kind="ExternalInput").ap()
    y = nc.dram_tensor("y", [HPC * 256, D], u8, kind="ExternalOutput").ap()

    with tile.TileContext(nc) as tc, ExitStack() as ctx:
        pool = lambda name, bufs, space="SBUF": ctx.enter_context(
            tc.tile_pool(name=name, bufs=bufs, space=space)
        )
        persist = pool("persist", 1)      # distinct tags -> own slots
        s_pool = pool("s_pool", 3)        # masked S tiles (fp16)
        t_pool = pool("t_pool", 4)        # spike-chain temporaries
        m_pool = pool("m_pool", 2)        # M snapshots (block-diag masked)
        y_pool = pool("y_pool", 3)        # output staging
        pp = pool("pp", 2, "PSUM")        # projections + final (fp32)
        ps = pool("ps", 1, "PSUM")        # S tiles (fp16, 2 parity tags)
        po = pool("po", 2, "PSUM")        # O accumulators (fp16, pair tiles)
        pm = pool("pm", 1, "PSUM")        # persistent M state (fp16)
        pt_ps = pool("pt_ps", 1, "PSUM")  # transpose staging (fp16)

        def ptile(shape, dtype, *, name):
            return persist.tile(shape, dtype, name=name, tag=name)

        # ---- SBUF allocations -----------------------------------------
        qt_sb = ptile([P, T], f32r, name="qt_sb")
        # per-piece k/v tiles: cols = 512*c + j (all 4 contraction chunks of
        # one 512-wide t-piece).  One DMA issue per tile keeps the
        # dependency of each piece's compute narrow.
        PW = T // NPIECE
        kt_p = [ptile([P, KC * PW], f32r, name=f"kt_p{pc}") for pc in range(NPIECE)]
        vt_p = [ptile([P, KC * PW], f32r, name=f"vt_p{pc}") for pc in range(NPIECE)]
        wp_sb = ptile([P, WPACK_W], f32r, name="wp_sb")
        wx_sb = ptile([P, WPACKX_W], f32, name="wx_sb")
        wk_sb = [wp_sb[:, OFF_WK + 256 * c :][:, 0:DH] for c in range(KC)]
        wv_sb = [wp_sb[:, OFF_WV + 256 * c :][:, 0:DH] for c in range(KC)]
        wq_sb = wp_sb[:, OFF_WQ : OFF_WQ + DH]
        wo_sb = [
            wx_sb[:, XOFF_WO + 256 * c : XOFF_WO + 256 * (c + 1)].bitcast(f16)
            for c in range(KC)
        ]
        msk_sb = wx_sb[:, XOFF_MSK : XOFF_MSK + 2 * P]               # [128,256]
        mdg_sb = wx_sb[:, XOFF_MDG : XOFF_MDG + P]                   # [128,128]
        bias_sb = wp_sb[:, OFF_BIAS:WPACK_W]
        bo_sb = wx_sb[:, XOFF_BO : XOFF_BO + 2 * P].bitcast(f16)     # [128,512]
        ones_sb = ptile([1, D], f32r, name="ones_sb")
        idt_sb = ptile([P, P], f16, name="idt_sb")
        idt_ones = ptile([1, P], f16, name="idt_ones")
        if has_bo:
            nc.vector.memset(idt_ones[:, :], 1.0)
        # qs/ks: spiked projections, d-major [dk, T]; tile i holds heads
        # 2i (parts 0:64) and 2i+1 (parts 64:128).
        qs = [ptile([P, T], f16, name=f"qs{i}") for i in range(2)]
        ks = [ptile([P, T], f16, name=f"ks{i}") for i in range(2)]
        # vkn: t-major spiked v for all 4 heads (cols 256t + 64*hl), fp16.
        vkn = ptile([P, DH * NT], f16, name="vkn")
        # kn: t-major spiked k, pair-major: cols 256t + 128*pair + 64*(hl%2)
        kn = ptile([P, DH * NT], f16, name="kn")
        # xs: spiked attention output, xs[p, 256*tt + 128*i + 64*sub + d]
        # (head h = 2i+sub): both the per-block spike write and the final
        # projection lhsT are fully contiguous [128,128] views.
        xs = ptile([P, 1024 * HPC], f16, name="xs")

        # ---- loads ----------------------------------------------------
        # The DMA ring fair-shares bandwidth across in-flight transfers;
        # tiny gate-copies (read prev dest, write next dest) order them.
        # Two parallel HWDGE FIFO rings: sync (qSPDynamicHW) carries wpk,
        # qt, vt and wpx; scalar (qActDynamicHW) carries kt.  Each piece's
        # transfers are issued JUST BEFORE the compute that consumes the
        # previous piece, so completion-semaphore waits never cover
        # later-issued transfers.
        ktv = kT.rearrange("(c p) t -> p c t", c=KC)
        vtv = vT.rearrange("(c p) t -> p c t", c=KC)

        def load_piece(pc):
            if pc > 0:
                gate(kt_p[pc - 1][0:1, 0:1], kt_p[pc][0:1, 0:1])
            nc.scalar.dma_start(
                out=kt_p[pc].rearrange("p (c j) -> p c j", c=KC),
                in_=ktv[:, :, ts(pc, PW)],
            )
            if pc > 0:
                gate(vt_p[pc - 1][0:1, 0:1], vt_p[pc][0:1, 0:1])
            nc.sync.dma_start(
                out=vt_p[pc].rearrange("p (c j) -> p c j", c=KC),
                in_=vtv[:, :, ts(pc, PW)],
            )

        # The DMA engines fair-share across ALL in-flight transfers, so
        # without ordering every transfer completes near the END of the
        # load phase.  Tiny gate-copies (read the previous transfer's
        # destination, write the next one's) serialize each ring so early
        # pieces finish early and compute can start at ~7us.
        def gate(prev_ap, next_ap):
            nc.vector.tensor_copy(next_ap, prev_ap)

        # First wave (ungated, all concurrent): everything piece-0 compute
        # needs.  Later pieces gate behind their predecessor so the DMA
        # engines' fair-sharing cannot starve the early pieces.
        nc.sync.dma_start(out=wp_sb[:, :], in_=wpk[:, :])
        nc.sync.dma_start(out=qt_sb[: DIN + 1, :], in_=qT[:, :])
        load_piece(0)
        gate(qt_sb[0:1, 0:1].bitcast(f32), wx_sb[0:1, 0:1])
        nc.sync.dma_start(out=wx_sb[:, :], in_=wpx[:, :])
        nc.vector.memset(ones_sb[:, :].bitcast(f32), 1.0)
        masks.make_identity(nc, idt_sb[:, :])

        BIG = float(2 ** 26)

        def spike_chain(out_ap, in_ap, nm):
            """spike(x) via two exact Relu ops on the ACT engine."""
            tmp = t_pool.tile([P, in_ap.free_size()],
                              f32, name=f"tmp_{nm}", tag=f"tmp_{nm}")
            nc.scalar.activation(tmp[:, :], in_ap, AF.Relu, bias=1.0, scale=-1.0)
            nc.scalar.activation(out_ap, tmp[:, :], AF.Relu, bias=1.0, scale=-BIG)

        # ---- per-piece projections ------------------------------------
        def q_piece(pc):
            for half in range(2):
                pt = pp.tile([P, 512], f32, name="pt", tag="pt")
                nc.tensor.matmul(
                    pt[:, :],
                    lhsT=wq_sb[: DIN + 1, ts(half, P)],
                    rhs=qt_sb[: DIN + 1, ts(pc, 512)],
                    start=True,
                    stop=True,
                )
                spike_chain(qs[half][:, ts(pc, 512)], pt[:, :], "q")

        def ks_chunk(ch):
            for half in range(2):
                pt = pp.tile([P, 512], f32, name="pt", tag="pt")
                for c in range(KC):
                    nc.tensor.matmul(
                        pt[:, :],
                        lhsT=wk_sb[c][:, ts(half, P)],
                        rhs=kt_p[ch][:, ts(c, PW)],
                        start=(c == 0),
                        stop=(c == KC - 1) and not has_bk,
                    )
                if has_bk:
                    nc.tensor.matmul(
                        pt[:, :],
                        lhsT=bias_sb[0:1, ts(half, P)],
                        rhs=ones_sb[0:1, 0:512],
                        start=False,
                        stop=True,
                    )
                spike_chain(ks[half][:, ts(ch, 512)], pt[:, :], "k")
            # t-major spiked K via PE transpose; both pair tiles land in one
            # PSUM tile so a single DVE copy moves them (kn block is
            # contiguous in the pair-major layout the M-update wants).
            for tt in range(4 * ch, 4 * ch + 4):
                tp = pt_ps.tile([P, 2 * P], f16, name="tp", tag="tp")
                for pr in range(2):
                    nc.tensor.transpose(
                        tp[:, ts(pr, P)], ks[pr][:, ts(tt, P)], idt_sb[:, :]
                    )
                nc.vector.tensor_copy(kn[:, ts(tt, DH)], tp[:, :])

        def vkn_block(tt):
            pt = pp.tile([P, 512], f32, name="pt", tag="pt")
            for c in range(KC):
                nc.tensor.matmul(
                    pt[:, 0:DH],
                    lhsT=vt_p[tt // 4][:, PW * c + P * (tt % 4) :][:, 0:P],
                    rhs=wv_sb[c][:, :],
                    start=(c == 0),
                    stop=(c == KC - 1) and not has_bv,
                )
            if has_bv:
                nc.tensor.matmul(
                    pt[:, 0:DH],
                    lhsT=ones_sb[0:1, 0:P],
                    rhs=bias_sb[1:2, 0:DH],
                    start=False,
                    stop=True,
                )
            spike_chain(vkn[:, ts(tt, DH)], pt[:, 0:DH], "v")

        # ---- attention ------------------------------------------------
        pm_t = pm.tile([P, DH], f32, name="pm_t")

        def attn_block(tt):
            if tt > 0:
                # snapshot M_(<tt), block-diag masked so the merged
                # per-pair O_inter matmul sees zero cross-head terms.
                m_sb = [
                    m_pool.tile([P, P], f16, name=f"m_sb{i}", tag=f"m_sb{i}")
                    for i in range(2)
                ]
                for i in range(2):
                    nc.vector.tensor_tensor(
                        m_sb[i][:, :], pm_t[:, ts(i, P)], mdg_sb[:, :],
                        op=ALU.mult,
                    )
            else:
                m_sb = None
            s_ps = [
                ps.tile([P, DH], f32, name=f"s_ps{par}", tag=f"s_ps{par}")
                for par in range(2)
            ]
            for hl in range(HPC):
                par, idx = hl % 2, hl // 2
                rows = slice(64 * par, 64 * par + 64)
                nc.tensor.matmul(
                    s_ps[par][:, ts(idx, P)],
                    lhsT=ks[idx][rows, ts(tt, P)],
                    rhs=qs[idx][rows, ts(tt, P)],
                    start=True,
                    stop=True,
                )
            s_sb = [
                s_pool.tile([P, DH], f16, name=f"s_sb{par}", tag=f"s_sb{par}")
                for par in range(2)
            ]
            for par in range(2):
                nc.vector.tensor_tensor(
                    s_sb[par][:, :], s_ps[par][:, :], msk_sb[:, :], op=ALU.mult
                )
            # o_ps[i]: [tq, dv of heads 2i (cols 0:64), 2i+1 (64:128)]
            # The two intra matmuls share ONE start: start=True clears the
            # has_written bits of the whole PSUM zero region, so a second
            # start would make the full-width O_inter OVERWRITE (not
            # accumulate onto) the first head's columns.
            o_ps = [po.tile([P, P], f32, name="o_ps") for _ in range(2)]
            for i in range(2):
                for par in range(2):
                    hl = 2 * i + par
                    nc.tensor.matmul(
                        o_ps[i][:, ts(par, 64)],
                        lhsT=s_sb[par][:, ts(i, P)],
                        rhs=vkn[:, DH * tt + 64 * hl :][:, 0:64],
                        start=(par == 0),
                        stop=(tt == 0),
                        skip_group_check=True,
                    )
            if tt > 0:
                for i in range(2):
                    nc.tensor.matmul(
                        o_ps[i][:, :],
                        lhsT=qs[i][:, ts(tt, P)],
                        rhs=m_sb[i][:, :],
                        start=False,
                        stop=True,
                        skip_group_check=True,
                    )
            # M += K_pair^T V_pair: one K=128,N=128 matmul per head pair;
            # only the diagonal 64x64 blocks are meaningful (snapshot is
            # masked).  stop=True closes the group so the snapshot read is
            # legal; on HW stop is a no-op and start=False keeps summing.
            for pr in range(2):
                nc.tensor.matmul(
                    pm_t[:, ts(pr, P)],
                    lhsT=kn[:, DH * tt + P * pr :][:, 0:P],
                    rhs=vkn[:, DH * tt + P * pr :][:, 0:P],
                    start=(tt == 0 and pr == 0),
                    stop=(pr == 1),
                    skip_group_check=True,
                )
            # x = spike(scale * O) = (O >= 1/scale): one DVE op per pair.
            for i in range(2):
                nc.vector.tensor_scalar(
                    xs[:, DH * tt + P * i :][:, 0:P],
                    o_ps[i][:, :],
                    inv_scale,
                    None,
                    ALU.is_ge,
                )

        # ---- final projection (fp16: xs {0,1} x Wo fp16) --------------
        # Output rows r with r%4 == m contract only over attention piece m
        # (X[r, f] = x_att[t=512*(r%4)+f, d=r//4]).  With the xs layout the
        # lhsT for (piece m, chunk cc, pair j) is the contiguous block
        # xs[:, 256*(4m+cc) + 128j :][:128].

        def final_piece(m):
            for j in range(2):  # head pair: heads 2j, 2j+1
                yp = pp.tile([P, 512], f32, name="pt", tag="pt")
                for cc in range(KC):
                    nc.tensor.matmul(
                        yp[:, :],
                        lhsT=xs[:, DH * (4 * m + cc) + P * j :][:, 0:P],
                        rhs=wo_sb[cc][:, :],
                        start=(cc == 0),
                        stop=(cc == KC - 1) and not has_bo,
                    )
                if has_bo:
                    nc.tensor.matmul(
                        yp[:, :],
                        lhsT=idt_ones[0:1, 0:P],
                        rhs=bo_sb[0:1, 0:512],
                        start=False,
                        stop=True,
                    )
                y_sb = y_pool.tile([P, D], u8, name="y_sb")
                nc.vector.tensor_scalar(
                    y_sb[:, :], yp[:, :], 1.0, None, ALU.is_ge
                )
                for sub in range(2):
                    h = 2 * j + sub
                    nc.gpsimd.dma_start(
                        out=y[256 * h + m : 256 * (h + 1) : 4, :],
                        in_=y_sb[64 * sub : 64 * sub + 64, :],
                    )

        # ---- schedule -------------------------------------------------
        def proj_piece(pc):
            q_piece(pc)
            ks_chunk(pc)
            for tt in range(4 * pc, 4 * pc + 4):
                vkn_block(tt)

        load_piece(1)
        proj_piece(0)
        load_piece(2)
        proj_piece(1)
        for pc in range(4):
            if pc + 2 < 4:
                if pc + 3 < 4:
                    load_piece(pc + 3)
                proj_piece(pc + 2)
            for tt in range(4 * pc, 4 * pc + 4):
                attn_block(tt)
            final_piece(pc)

    nc.compile()
    return nc


def _get_prog(scale, has_bk, has_bv, has_bo):
    key = (scale, has_bk, has_bv, has_bo)
    if key not in _prog_cache:
        _prog_cache[key] = _build(scale, has_bk, has_bv, has_bo)
    return _prog_cache[key]


def _pack_weights(Wq, bq, Wk, bk, Wv, bv, Wo, bo, cs):
    wpk = np.zeros((P, WPACK_W), np.float32)
    wpx = np.zeros((P, WPACKX_W), np.float32)
    for c in range(KC):
        wpk[:, OFF_WK + 256 * c : OFF_WK + 256 * (c + 1)] = _f32r_round(
            Wk[128 * c : 128 * (c + 1), cs]
        )
        wpk[:, OFF_WV + 256 * c : OFF_WV + 256 * (c + 1)] = _f32r_round(
            Wv[128 * c : 128 * (c + 1), cs]
        )
        # Wo fp16 pairs packed into f32 words
        wo16 = np.ascontiguousarray(
            Wo[128 * c : 128 * (c + 1), :].astype(np.float16)
        )
        wpx[:, XOFF_WO + 256 * c : XOFF_WO + 256 * (c + 1)] = wo16.view(
            np.float32
        )
    wq = np.zeros((P, DH), np.float32)
    wq[:DIN] = Wq[:, cs]
    wq[DIN] = bq[cs]
    wpk[:, OFF_WQ : OFF_WQ + DH] = _f32r_round(wq)
    wpx[:, XOFF_MSK : XOFF_MSK + 2 * P] = np.tile(
        np.triu(np.ones((P, P), np.float32)), (1, 2)
    )
    wpx[:, XOFF_MDG : XOFF_MDG + P] = np.kron(
        np.eye(2, dtype=np.float32), np.ones((64, 64), np.float32)
    )
    wpk[0, OFF_BIAS : OFF_BIAS + DH] = _f32r_round(
        np.ascontiguousarray(bk[cs], np.float32)
    )
    wpk[1, OFF_BIAS : OFF_BIAS + DH] = _f32r_round(
        np.ascontiguousarray(bv[cs], np.float32)
    )
    bo16 = np.ascontiguousarray(bo.astype(np.float16))
    wpx[0, XOFF_BO : XOFF_BO + D // 2] = bo16.view(np.float32)
    return wpk, wpx


def kernel(**inputs) -> np.ndarray:
    global last_exec_time_ns
    from concourse.bass_utils import run_bass_kernel_spmd

    g = lambda n: np.asarray(inputs[n], dtype=np.float32)
    query, key, value = g("query"), g("key"), g("value")
    Wq, bq, Wk, bk = g("Wq"), g("bq"), g("Wk"), g("bk")
    Wv, bv, Wo, bo = g("Wv"), g("bv"), g("Wo"), g("bo")
    scale = float(np.asarray(inputs["scale"], dtype=np.float32).reshape(-1)[0])

    has_bk, has_bv, has_bo = (bool(np.any(x)) for x in (bk, bv, bo))
    prog = _get_prog(scale, has_bk, has_bv, has_bo)

    # pre-round the shared per-batch data once
    qTr, kTr, vTr = [None] * B, [None] * B, [None] * B
    for b in range(B):
        qTa = np.empty((DIN + 1, T), np.float32)
        qTa[:DIN] = _f32r_round(np.ascontiguousarray(query[b].T))
        qTa[DIN] = 1.0
        qTr[b] = qTa
        kTr[b] = _f32r_round(np.ascontiguousarray(key[b].T))
        vTr[b] = _f32r_round(np.ascontiguousarray(value[b].T))

    in_maps = []
    for c in range(NCORES):
        b, hg = divmod(c, 2)
        cs = slice(DH * hg, DH * (hg + 1))
        wpk_c, wpx_c = _pack_weights(Wq, bq, Wk, bk, Wv, bv, Wo, bo, cs)
        in_maps.append(
            {
                "qT": qTr[b],
                "kT": kTr[b],
                "vT": vTr[b],
                "wpk": wpk_c,
                "wpx": wpx_c,
            }
        )

    trace = os.environ.get("BASS_TRACE", "") not in ("", "0")
    res = run_bass_kernel_spmd(
        prog, in_maps, core_ids=list(range(NCORES)), trace=trace
    )
    last_exec_time_ns = res.exec_time_ns
    if res.exec_time_ns is not None:
        print(f"HW exec time: {res.exec_time_ns} ns")

    out = np.empty((B, T, D), np.float32)
    for c in range(NCORES):
        b, hg = divmod(c, 2)
        out[b, 1024 * hg : 1024 * (hg + 1)] = res.results[c]["y"].astype(
            np.float32
        )
    return out


# revision 18
# speedup vs baseline: 1.0458x; 1.0136x over previous
"""Trainium2 Bass kernel: spiking multi-head attention (nn_MultiHeadedAttention).

Reference semantics (B=4, T=2048, DIN=100, D=512, h=8 heads, dk=64):
    q = spike(query @ Wq + bq)   (spike = (x >= 1.0) -> {0,1})
    k = spike(key @ Wk + bk);  v = spike(value @ Wv + bv)
    attn = (q @ k^T) * scale, causally masked (keep k<=q), NO softmax
    x = spike(attn @ v)
    x = x.transpose(0,1,3,2).reshape(B,T,h*dk)    # scrambled reshape
    y = spike(x @ Wo + bo)

Key facts exploited:
  * No softmax -> causal attention is LINEAR attention:
        O_t = q_t . M_t  +  intra-block tril(Q K^T) V,   M = sum_j k_j v_j^T
    so only the 16 diagonal 128x128 S-tiles per head are materialized.
  * The scrambled reshape maps output rows [256*h, 256*(h+1)) to exactly one
    head h, so head-parallel sharding needs NO cross-core communication.
  * float32r matmuls stream 1 col/cycle for N>=256 (4x fp32, measured 227ns
    vs 860ns at N=512) with fp32-range 12-bit-mantissa operands; the host
    pre-rounds all real-valued operands to the f32r grid so every projection
    runs at bf16 speed.  End-to-end spike-flip error from the rounding is
    ~1.3e-2 rel (gate 2e-2), verified against the reference on CPU.
  * Spiked tensors are {0,1} and S / O / M are integers, all exact in fp16;
    the attention core uses fp16 operands AND fp16 PSUM tiles (integer
    values -> exact; halves DVE PSUM-read cost and PSUM bank usage).
  * O_inter is ONE matmul per head pair (lhsT = full 128-row qs tile times a
    block-diag-masked M snapshot), halving its LDWEIGHTS cost.
  * Final projection: xs {0,1} fp16 x Wo fp16 (adds ~51 spike flips), y
    spike emitted as uint8 (host upcasts) cutting output DMA 4x.

Sharding: core c -> batch b=c//2, head-group hg=c%2 (4 heads per core).

Hardware notes encoded below:
  * K=64 matmuls whose lhsT sits at partition base 0 vs base 64 execute
    concurrently in disjoint PE row groups; concurrent writes to one PSUM
    bank hang the device -> parity-0/1 S tiles live in different banks
    (PSUM pool slots are bank-padded, one slot per tag).
  * PSUM budget is exactly 8 banks: pp(2, fp32 projections+final) ps(2,
    fp16 S parities) po(2, fp16 O pairs) pm(1, fp16 M state) tp(1, fp16
    transposes).
  * Engine balance: DVE does the tensor_tensor masks + y/kn; ACT+GPSIMD
    run the two-op spike chains (GPSIMD cannot read PSUM, so it always
    takes the SBUF-side second op).
  * DMA-issue instructions are expensive on the issuing engine; all weights
    ride one packed f32r tensor and k/v loads are paced with gate-copies so
    early-needed pieces get full ring bandwidth.
"""

import os
import numpy as np

B, T, DIN, D = 4, 2048, 100, 512
H, DK = 8, 64
NCORES = 8
HPC = 4          # heads per core
DH = HPC * DK    # 256 projected features per core
P = 128
NT = T // P      # 16 t-blocks
KC = D // P      # 4 contraction chunks of the D=512 dim
NPIECE = 4       # pieces along T (512 each)

# packed weights ride in two tensors: wpk (f32r: everything consumed by
# f32r matmuls -- the compiler re-rounds f32r DRAM data, so bit-packed f16
# payloads must NOT live there) and wpx (plain f32: Wo f16 pairs + masks).
OFF_WK = 0                      # 4 chunks x 256 = 1024 f32r cols
OFF_WV = 1024                   # 1024 cols
OFF_WQ = 2048                   # 256 cols
OFF_BIAS = 2304                 # rows 0/1 = bk/bv (f32r)
WPACK_W = 2560                  # f32r tensor width
XOFF_WO = 0                     # Wo fp16 pairs: 4 chunks x 512 f16 = 1024 f32 cols
XOFF_MSK = 1024                 # triu mask f32: 256 cols
XOFF_MDG = 1280                 # block-diag mask f32: 128 cols
XOFF_BO = 1408                  # row 0 = bo (fp16 pairs, 256 f32 cols)
WPACKX_W = 1664                 # f32 tensor width

_prog_cache: dict = {}
last_exec_time_ns = None


def _f32r_round(x: np.ndarray) -> np.ndarray:
    """Round fp32 to the f32r grid (11 explicit mantissa bits, RN)."""
    u = np.ascontiguousarray(x, np.float32).view(np.uint32)
    r = (u + np.uint32(0x7FF) + ((u >> np.uint32(12)) & np.uint32(1))) & np.uint32(
        0xFFFFF000
    )
    return r.view(np.float32)


def _build(scale: float, has_bk: bool, has_bv: bool, has_bo: bool):
    from contextlib import ExitStack

    import concourse.bass as bass
    import concourse.tile as tile
    import concourse.mybir as mybir
    from concourse import bacc
    from concourse.bass import ts
    from concourse import masks

    f32 = mybir.dt.float32
    f32r = mybir.dt.float32r
    f16 = mybir.dt.float16
    u8 = mybir.dt.uint8
    ALU = mybir.AluOpType
    AF = mybir.ActivationFunctionType
    inv_scale = 1.0 / scale

    nc = bacc.Bacc(
        "TRN2", target_bir_lowering=False, debug=False, num_devices=NCORES
    )

    # DRAM I/O (host pre-transposes and pre-rounds to the f32r grid; qT
    # carries an extra all-ones row so bq rides in Wq's last row).
    qT = nc.dram_tensor("qT", [DIN + 1, T], f32r, kind="ExternalInput").ap()
    # kvT[p, 4096*pc + u]: u<2048 -> kt piece pc (cols 512c+j), u>=2048 ->
    # vt piece pc.  Host pre-arranges so every piece is ONE fully
    # contiguous transfer (contiguous DMA measured 425GB/s vs 187GB/s for
    # the strided multi-chunk AP; single-ring FIFO avoids the ~4x loss
    # from concurrent-ring thrash).
    kvT = nc.dram_tensor("kvT", [P, 8 * T], f32r, kind="ExternalInput").ap()
    wpk = nc.dram_tensor("wpk", [P, WPACK_W], f32r, kind="ExternalInput").ap()
    wpx = nc.dram_tensor("wpx", [P, WPACKX_W], f32, kind="ExternalInput").ap()
    y = nc.dram_tensor("y", [HPC * 256, D], u8, kind="ExternalOutput").ap()

    with tile.TileContext(nc) as tc, ExitStack() as ctx:
        pool = lambda name, bufs, space="SBUF": ctx.enter_context(
            tc.tile_pool(name=name, bufs=bufs, space=space)
        )
        persist = pool("persist", 1)      # distinct tags -> own slots
        s_pool = pool("s_pool", 3)        # masked S tiles (fp16)
        t_pool = pool("t_pool", 4)        # spike-chain temporaries
        m_pool = pool("m_pool", 2)        # M snapshots (block-diag masked)
        y_pool = pool("y_pool", 3)        # output staging
        pp = pool("pp", 2, "PSUM")        # projections + final (fp32)
        ps = pool("ps", 1, "PSUM")        # S tiles (fp16, 2 parity tags)
        po = pool("po", 2, "PSUM")        # O accumulators (fp16, pair tiles)
        pm = pool("pm", 1, "PSUM")        # persistent M state (fp16)
        pt_ps = pool("pt_ps", 1, "PSUM")  # transpose staging (fp16)

        def ptile(shape, dtype, *, name):
            return persist.tile(shape, dtype, name=name, tag=name)

        # ---- SBUF allocations -----------------------------------------
        qt_sb = ptile([P, T], f32r, name="qt_sb")
        # per-piece k/v tiles: one contiguous [128, 4096] tile per piece
        # (kt cols 0:2048 = 512*c+j, vt cols 2048:4096).
        PW = T // NPIECE
        kv_p = [ptile([P, 2 * KC * PW], f32r, name=f"kv_p{pc}") for pc in range(NPIECE)]
        kt_p = [kv_p[pc][:, 0 : KC * PW] for pc in range(NPIECE)]
        vt_p = [kv_p[pc][:, KC * PW : 2 * KC * PW] for pc in range(NPIECE)]
        wp_sb = ptile([P, WPACK_W], f32r, name="wp_sb")
        wx_sb = ptile([P, WPACKX_W], f32, name="wx_sb")
        wk_sb = [wp_sb[:, OFF_WK + 256 * c :][:, 0:DH] for c in range(KC)]
        wv_sb = [wp_sb[:, OFF_WV + 256 * c :][:, 0:DH] for c in range(KC)]
        wq_sb = wp_sb[:, OFF_WQ : OFF_WQ + DH]
        wo_sb = [
            wx_sb[:, XOFF_WO + 256 * c : XOFF_WO + 256 * (c + 1)].bitcast(f16)
            for c in range(KC)
        ]
        msk_sb = wx_sb[:, XOFF_MSK : XOFF_MSK + 2 * P]               # [128,256]
        mdg_sb = wx_sb[:, XOFF_MDG : XOFF_MDG + P]                   # [128,128]
        bias_sb = wp_sb[:, OFF_BIAS:WPACK_W]
        bo_sb = wx_sb[:, XOFF_BO : XOFF_BO + 2 * P].bitcast(f16)     # [128,512]
        ones_sb = ptile([1, D], f32r, name="ones_sb")
        idt_sb = ptile([P, P], f16, name="idt_sb")
        idt_ones = ptile([1, P], f16, name="idt_ones")
        if has_bo:
            nc.vector.memset(idt_ones[:, :], 1.0)
        # qs/ks: spiked projections, d-major [dk, T]; tile i holds heads
        # 2i (parts 0:64) and 2i+1 (parts 64:128).
        qs = [ptile([P, T], f16, name=f"qs{i}") for i in range(2)]
        ks = [ptile([P, T], f16, name=f"ks{i}") for i in range(2)]
        # vkn: t-major spiked v for all 4 heads (cols 256t + 64*hl), fp16.
        vkn = ptile([P, DH * NT], f16, name="vkn")
        # kn: t-major spiked k, pair-major: cols 256t + 128*pair + 64*(hl%2)
        kn = ptile([P, DH * NT], f16, name="kn")
        # xs: spiked attention output, xs[p, 256*tt + 128*i + 64*sub + d]
        # (head h = 2i+sub): both the per-block spike write and the final
        # projection lhsT are fully contiguous [128,128] views.
        xs = ptile([P, 1024 * HPC], f16, name="xs")

        # ---- loads ----------------------------------------------------
        # The DMA ring fair-shares bandwidth across in-flight transfers;
        # tiny gate-copies (read prev dest, write next dest) order them.
        # Two parallel HWDGE FIFO rings: sync (qSPDynamicHW) carries wpk,
        # qt, vt and wpx; scalar (qActDynamicHW) carries kt.  Each piece's
        # transfers are issued JUST BEFORE the compute that consumes the
        # previous piece, so completion-semaphore waits never cover
        # later-issued transfers.
        def load_piece(pc):
            nc.sync.dma_start(
                out=kv_p[pc][:, :], in_=kvT[:, ts(pc, 2 * KC * PW)]
            )

        # Single sync ring, FIFO, priority order; all transfers contiguous.
        nc.sync.dma_start(out=wp_sb[:, :], in_=wpk[:, :])
        nc.sync.dma_start(out=qt_sb[: DIN + 1, :], in_=qT[:, :])
        load_piece(0)
        nc.sync.dma_start(out=wx_sb[:, :], in_=wpx[:, :])
        nc.vector.memset(ones_sb[:, :].bitcast(f32), 1.0)
        masks.make_identity(nc, idt_sb[:, :])

        BIG = float(2 ** 26)

        def spike_chain(out_ap, in_ap, nm):
            """spike(x) via two exact Relu ops on the ACT engine."""
            tmp = t_pool.tile([P, in_ap.free_size()],
                              f32, name=f"tmp_{nm}", tag=f"tmp_{nm}")
            nc.scalar.activation(tmp[:, :], in_ap, AF.Relu, bias=1.0, scale=-1.0)
            nc.scalar.activation(out_ap, tmp[:, :], AF.Relu, bias=1.0, scale=-BIG)

        # ---- per-piece projections ------------------------------------
        def q_piece(pc):
            for half in range(2):
                pt = pp.tile([P, 512], f32, name="pt", tag="pt")
                nc.tensor.matmul(
                    pt[:, :],
                    lhsT=wq_sb[: DIN + 1, ts(half, P)],
                    rhs=qt_sb[: DIN + 1, ts(pc, 512)],
                    start=True,
                    stop=True,
                )
                spike_chain(qs[half][:, ts(pc, 512)], pt[:, :], "q")

        def ks_chunk(ch):
            for half in range(2):
                pt = pp.tile([P, 512], f32, name="pt", tag="pt")
                for c in range(KC):
                    nc.tensor.matmul(
                        pt[:, :],
                        lhsT=wk_sb[c][:, ts(half, P)],
                        rhs=kt_p[ch][:, ts(c, PW)],
                        start=(c == 0),
                        stop=(c == KC - 1) and not has_bk,
                    )
                if has_bk:
                    nc.tensor.matmul(
                        pt[:, :],
                        lhsT=bias_sb[0:1, ts(half, P)],
                        rhs=ones_sb[0:1, 0:512],
                        start=False,
                        stop=True,
                    )
                spike_chain(ks[half][:, ts(ch, 512)], pt[:, :], "k")
            # t-major spiked K via PE transpose; both pair tiles land in one
            # PSUM tile so a single DVE copy moves them (kn block is
            # contiguous in the pair-major layout the M-update wants).
            for tt in range(4 * ch, 4 * ch + 4):
                tp = pt_ps.tile([P, 2 * P], f16, name="tp", tag="tp")
                for pr in range(2):
                    nc.tensor.transpose(
                        tp[:, ts(pr, P)], ks[pr][:, ts(tt, P)], idt_sb[:, :]
                    )
                nc.vector.tensor_copy(kn[:, ts(tt, DH)], tp[:, :])

        def vkn_block(tt):
            pt = pp.tile([P, 512], f32, name="pt", tag="pt")
            for c in range(KC):
                nc.tensor.matmul(
                    pt[:, 0:DH],
                    lhsT=vt_p[tt // 4][:, PW * c + P * (tt % 4) :][:, 0:P],
                    rhs=wv_sb[c][:, :],
                    start=(c == 0),
                    stop=(c == KC - 1) and not has_bv,
                )
            if has_bv:
                nc.tensor.matmul(
                    pt[:, 0:DH],
                    lhsT=ones_sb[0:1, 0:P],
                    rhs=bias_sb[1:2, 0:DH],
                    start=False,
                    stop=True,
                )
            spike_chain(vkn[:, ts(tt, DH)], pt[:, 0:DH], "v")

        # ---- attention ------------------------------------------------
        pm_t = pm.tile([P, DH], f32, name="pm_t")

        def attn_block(tt):
            if tt > 0:
                # snapshot M_(<tt), block-diag masked so the merged
                # per-pair O_inter matmul sees zero cross-head terms.
                m_sb = [
                    m_pool.tile([P, P], f16, name=f"m_sb{i}", tag=f"m_sb{i}")
                    for i in range(2)
                ]
                for i in range(2):
                    nc.vector.tensor_tensor(
                        m_sb[i][:, :], pm_t[:, ts(i, P)], mdg_sb[:, :],
                        op=ALU.mult,
                    )
            else:
                m_sb = None
            s_ps = [
                ps.tile([P, DH], f32, name=f"s_ps{par}", tag=f"s_ps{par}")
                for par in range(2)
            ]
            for hl in range(HPC):
                par, idx = hl % 2, hl // 2
                rows = slice(64 * par, 64 * par + 64)
                nc.tensor.matmul(
                    s_ps[par][:, ts(idx, P)],
                    lhsT=ks[idx][rows, ts(tt, P)],
                    rhs=qs[idx][rows, ts(tt, P)],
                    start=True,
                    stop=True,
                )
            s_sb = [
                s_pool.tile([P, DH], f16, name=f"s_sb{par}", tag=f"s_sb{par}")
                for par in range(2)
            ]
            for par in range(2):
                nc.vector.tensor_tensor(
                    s_sb[par][:, :], s_ps[par][:, :], msk_sb[:, :], op=ALU.mult
                )
            # o_ps[i]: [tq, dv of heads 2i (cols 0:64), 2i+1 (64:128)]
            # The two intra matmuls share ONE start: start=True clears the
            # has_written bits of the whole PSUM zero region, so a second
            # start would make the full-width O_inter OVERWRITE (not
            # accumulate onto) the first head's columns.
            o_ps = [po.tile([P, P], f32, name="o_ps") for _ in range(2)]
            for i in range(2):
                for par in range(2):
                    hl = 2 * i + par
                    nc.tensor.matmul(
                        o_ps[i][:, ts(par, 64)],
                        lhsT=s_sb[par][:, ts(i, P)],
                        rhs=vkn[:, DH * tt + 64 * hl :][:, 0:64],
                        start=(par == 0),
                        stop=(tt == 0),
                        skip_group_check=True,
                    )
            if tt > 0:
                for i in range(2):
                    nc.tensor.matmul(
                        o_ps[i][:, :],
                        lhsT=qs[i][:, ts(tt, P)],
                        rhs=m_sb[i][:, :],
                        start=False,
                        stop=True,
                        skip_group_check=True,
                    )
            # M += K_pair^T V_pair: one K=128,N=128 matmul per head pair;
            # only the diagonal 64x64 blocks are meaningful (snapshot is
            # masked).  stop=True closes the group so the snapshot read is
            # legal; on HW stop is a no-op and start=False keeps summing.
            for pr in range(2):
                nc.tensor.matmul(
                    pm_t[:, ts(pr, P)],
                    lhsT=kn[:, DH * tt + P * pr :][:, 0:P],
                    rhs=vkn[:, DH * tt + P * pr :][:, 0:P],
                    start=(tt == 0 and pr == 0),
                    stop=(pr == 1),
                    skip_group_check=True,
                )
            # x = spike(scale * O) = (O >= 1/scale): one DVE op per pair.
            for i in range(2):
                nc.vector.tensor_scalar(
                    xs[:, DH * tt + P * i :][:, 0:P],
                    o_ps[i][:, :],
                    inv_scale,
                    None,
                    ALU.is_ge,
                )

        # ---- final projection (fp16: xs {0,1} x Wo fp16) --------------
        # Output rows r with r%4 == m contract only over attention piece m
        # (X[r, f] = x_att[t=512*(r%4)+f, d=r//4]).  With the xs layout the
        # lhsT for (piece m, chunk cc, pair j) is the contiguous block
        # xs[:, 256*(4m+cc) + 128j :][:128].

        def final_piece(m):
            for j in range(2):  # head pair: heads 2j, 2j+1
                yp = pp.tile([P, 512], f32, name="pt", tag="pt")
                for cc in range(KC):
                    nc.tensor.matmul(
                        yp[:, :],
                        lhsT=xs[:, DH * (4 * m + cc) + P * j :][:, 0:P],
                        rhs=wo_sb[cc][:, :],
                        start=(cc == 0),
                        stop=(cc == KC - 1) and not has_bo,
                    )
                if has_bo:
                    nc.tensor.matmul(
                        yp[:, :],
                        lhsT=idt_ones[0:1, 0:P],
                        rhs=bo_sb[0:1, 0:512],
                        start=False,
                        stop=True,
                    )
                y_sb = y_pool.tile([P, D], u8, name="y_sb")
                nc.vector.tensor_scalar(
                    y_sb[:, :], yp[:, :], 1.0, None, ALU.is_ge
                )
                for sub in range(2):
                    h = 2 * j + sub
                    nc.gpsimd.dma_start(
                        out=y[256 * h + m : 256 * (h + 1) : 4, :],
                        in_=y_sb[64 * sub : 64 * sub + 64, :],
                    )

        # ---- schedule -------------------------------------------------
        def proj_piece(pc):
            q_piece(pc)
            ks_chunk(pc)
            for tt in range(4 * pc, 4 * pc + 4):
                vkn_block(tt)

        load_piece(1)
        proj_piece(0)
        load_piece(2)
        proj_piece(1)
        for pc in range(4):
            if pc + 2 < 4:
                if pc + 3 < 4:
                    load_piece(pc + 3)
                proj_piece(pc + 2)
            for tt in range(4 * pc, 4 * pc + 4):
                attn_block(tt)
            final_piece(pc)

    nc.compile()
    return nc


def _get_prog(scale, has_bk, has_bv, has_bo):
    key = (scale, has_bk, has_bv, has_bo)
    if key not in _prog_cache:
        _prog_cache[key] = _build(scale, has_bk, has_bv, has_bo)
    return _prog_cache[key]


def _pack_weights(Wq, bq, Wk, bk, Wv, bv, Wo, bo, cs):
    wpk = np.zeros((P, WPACK_W), np.float32)
    wpx = np.zeros((P, WPACKX_W), np.float32)
    for c in range(KC):
        wpk[:, OFF_WK + 256 * c : OFF_WK + 256 * (c + 1)] = _f32r_round(
            Wk[128 * c : 128 * (c + 1), cs]
        )
        wpk[:, OFF_WV + 256 * c : OFF_WV + 256 * (c + 1)] = _f32r_round(
            Wv[128 * c : 128 * (c + 1), cs]
        )
        # Wo fp16 pairs packed into f32 words
        wo16 = np.ascontiguousarray(
            Wo[128 * c : 128 * (c + 1), :].astype(np.float16)
        )
        wpx[:, XOFF_WO + 256 * c : XOFF_WO + 256 * (c + 1)] = wo16.view(
            np.float32
        )
    wq = np.zeros((P, DH), np.float32)
    wq[:DIN] = Wq[:, cs]
    wq[DIN] = bq[cs]
    wpk[:, OFF_WQ : OFF_WQ + DH] = _f32r_round(wq)
    wpx[:, XOFF_MSK : XOFF_MSK + 2 * P] = np.tile(
        np.triu(np.ones((P, P), np.float32)), (1, 2)
    )
    wpx[:, XOFF_MDG : XOFF_MDG + P] = np.kron(
        np.eye(2, dtype=np.float32), np.ones((64, 64), np.float32)
    )
    wpk[0, OFF_BIAS : OFF_BIAS + DH] = _f32r_round(
        np.ascontiguousarray(bk[cs], np.float32)
    )
    wpk[1, OFF_BIAS : OFF_BIAS + DH] = _f32r_round(
        np.ascontiguousarray(bv[cs], np.float32)
    )
    bo16 = np.ascontiguousarray(bo.astype(np.float16))
    wpx[0, XOFF_BO : XOFF_BO + D // 2] = bo16.view(np.float32)
    return wpk, wpx


def kernel(**inputs) -> np.ndarray:
    global last_exec_time_ns
    from concourse.bass_utils import run_bass_kernel_spmd

    g = lambda n: np.asarray(inputs[n], dtype=np.float32)
    query, key, value = g("query"), g("key"), g("value")
    Wq, bq, Wk, bk = g("Wq"), g("bq"), g("Wk"), g("bk")
    Wv, bv, Wo, bo = g("Wv"), g("bv"), g("Wo"), g("bo")
    scale = float(np.asarray(inputs["scale"], dtype=np.float32).reshape(-1)[0])

    has_bk, has_bv, has_bo = (bool(np.any(x)) for x in (bk, bv, bo))
    prog = _get_prog(scale, has_bk, has_bv, has_bo)

    # pre-round the shared per-batch data once; arrange kt/vt piece-major
    # so each piece is one contiguous DMA:
    #   kvT[p, 4096*pc + 512*c + j]        = kT[128c+p, 512pc+j]
    #   kvT[p, 4096*pc + 2048 + 512*c + j] = vT[128c+p, 512pc+j]
    qTr, kvTr = [None] * B, [None] * B
    for b in range(B):
        qTa = np.empty((DIN + 1, T), np.float32)
        qTa[:DIN] = _f32r_round(np.ascontiguousarray(query[b].T))
        qTa[DIN] = 1.0
        qTr[b] = qTa
        kk = _f32r_round(np.ascontiguousarray(key[b].T))
        vv = _f32r_round(np.ascontiguousarray(value[b].T))
        # [512, 2048] -> [pc, p, 512c+j]
        ka = kk.reshape(KC, P, NPIECE, 512).transpose(2, 1, 0, 3).reshape(
            NPIECE, P, KC * 512
        )
        va = vv.reshape(KC, P, NPIECE, 512).transpose(2, 1, 0, 3).reshape(
            NPIECE, P, KC * 512
        )
        kv = np.concatenate([ka, va], axis=2)  # [pc, p, 4096]
        kvTr[b] = np.ascontiguousarray(kv.transpose(1, 0, 2)).reshape(P, 8 * T)

    in_maps = []
    for c in range(NCORES):
        b, hg = divmod(c, 2)
        cs = slice(DH * hg, DH * (hg + 1))
        wpk_c, wpx_c = _pack_weights(Wq, bq, Wk, bk, Wv, bv, Wo, bo, cs)
        in_maps.append(
            {
                "qT": qTr[b],
                "kvT": kvTr[b],
                "wpk": wpk_c,
                "wpx": wpx_c,
            }
        )

    trace = os.environ.get("BASS_TRACE", "") not in ("", "0")
    res = run_bass_kernel_spmd(
        prog, in_maps, core_ids=list(range(NCORES)), trace=trace
    )
    last_exec_time_ns = res.exec_time_ns
    if res.exec_time_ns is not None:
        print(f"HW exec time: {res.exec_time_ns} ns")

    out = np.empty((B, T, D), np.float32)
    for c in range(NCORES):
        b, hg = divmod(c, 2)
        out[b, 1024 * hg : 1024 * (hg + 1)] = res.results[c]["y"].astype(
            np.float32
        )
    return out


# revision 19
# speedup vs baseline: 1.5238x; 1.4570x over previous
"""Trainium2 Bass kernel: spiking multi-head attention (nn_MultiHeadedAttention).

Reference semantics (B=4, T=2048, DIN=100, D=512, h=8 heads, dk=64):
    q = spike(query @ Wq + bq)   (spike = (x >= 1.0) -> {0,1})
    k = spike(key @ Wk + bk);  v = spike(value @ Wv + bv)
    attn = (q @ k^T) * scale, causally masked (keep k<=q), NO softmax
    x = spike(attn @ v)
    x = x.transpose(0,1,3,2).reshape(B,T,h*dk)    # scrambled reshape
    y = spike(x @ Wo + bo)

Key facts exploited:
  * No softmax -> causal attention is LINEAR attention:
        O_t = q_t . M_t  +  intra-block tril(Q K^T) V,   M = sum_j k_j v_j^T
    so only the 16 diagonal 128x128 S-tiles per head are materialized.
  * The scrambled reshape maps output rows [256*h, 256*(h+1)) to exactly one
    head h, so head-parallel sharding needs NO cross-core communication.
  * float32r matmuls stream 1 col/cycle for N>=256 (4x fp32, measured 227ns
    vs 860ns at N=512) with fp32-range 12-bit-mantissa operands; the host
    pre-rounds all real-valued operands to the f32r grid so every projection
    runs at bf16 speed.  End-to-end spike-flip error from the rounding is
    ~1.3e-2 rel (gate 2e-2), verified against the reference on CPU.
  * Spiked tensors are {0,1} and S / O / M are integers, all exact in fp16;
    the attention core uses fp16 operands AND fp16 PSUM tiles (integer
    values -> exact; halves DVE PSUM-read cost and PSUM bank usage).
  * O_inter is ONE matmul per head pair (lhsT = full 128-row qs tile times a
    block-diag-masked M snapshot), halving its LDWEIGHTS cost.
  * Final projection: xs {0,1} fp16 x Wo fp16 (adds ~51 spike flips), y
    spike emitted as uint8 (host upcasts) cutting output DMA 4x.

Sharding: core c -> batch b=c//2, head-group hg=c%2 (4 heads per core).

Hardware notes encoded below:
  * K=64 matmuls whose lhsT sits at partition base 0 vs base 64 execute
    concurrently in disjoint PE row groups; concurrent writes to one PSUM
    bank hang the device -> parity-0/1 S tiles live in different banks
    (PSUM pool slots are bank-padded, one slot per tag).
  * PSUM budget is exactly 8 banks: pp(2, fp32 projections+final) ps(2,
    fp16 S parities) po(2, fp16 O pairs) pm(1, fp16 M state) tp(1, fp16
    transposes).
  * Engine balance: DVE does the tensor_tensor masks + y/kn; ACT+GPSIMD
    run the two-op spike chains (GPSIMD cannot read PSUM, so it always
    takes the SBUF-side second op).
  * DMA-issue instructions are expensive on the issuing engine; all weights
    ride one packed f32r tensor and k/v loads are paced with gate-copies so
    early-needed pieces get full ring bandwidth.
"""

import os
import numpy as np

B, T, DIN, D = 4, 2048, 100, 512
H, DK = 8, 64
NCORES = 8
HPC = 4          # heads per core
DH = HPC * DK    # 256 projected features per core
P = 128
NT = T // P      # 16 t-blocks
KC = D // P      # 4 contraction chunks of the D=512 dim
NPIECE = 4       # pieces along T (512 each)

# packed weights ride in two tensors: wpk (f32r: everything consumed by
# f32r matmuls -- the compiler re-rounds f32r DRAM data, so bit-packed f16
# payloads must NOT live there) and wpx (plain f32: Wo f16 pairs + masks).
OFF_WK = 0                      # 4 chunks x 256 = 1024 f32r cols
OFF_WV = 1024                   # 1024 cols
OFF_WQ = 2048                   # 256 cols
OFF_BIAS = 2304                 # rows 0/1 = bk/bv (f32r)
WPACK_W = 2560                  # f32r tensor width
XOFF_WO = 0                     # Wo fp16 pairs: 4 chunks x 512 f16 = 1024 f32 cols
XOFF_MSK = 1024                 # triu mask f32: 256 cols
XOFF_MDG = 1280                 # block-diag mask f32: 128 cols
XOFF_BO = 1408                  # row 0 = bo (fp16 pairs, 256 f32 cols)
WPACKX_W = 1664                 # f32 tensor width

_prog_cache: dict = {}
last_exec_time_ns = None


def _f32r_round(x: np.ndarray) -> np.ndarray:
    """Round fp32 to the f32r grid (11 explicit mantissa bits, RN)."""
    u = np.ascontiguousarray(x, np.float32).view(np.uint32)
    r = (u + np.uint32(0x7FF) + ((u >> np.uint32(12)) & np.uint32(1))) & np.uint32(
        0xFFFFF000
    )
    return r.view(np.float32)


def _build(scale: float, has_bk: bool, has_bv: bool, has_bo: bool):
    from contextlib import ExitStack

    import concourse.bass as bass
    import concourse.tile as tile
    import concourse.mybir as mybir
    from concourse import bacc
    from concourse.bass import ts
    from concourse import masks

    f32 = mybir.dt.float32
    f32r = mybir.dt.float32r
    f16 = mybir.dt.float16
    u8 = mybir.dt.uint8
    ALU = mybir.AluOpType
    AF = mybir.ActivationFunctionType
    inv_scale = 1.0 / scale

    nc = bacc.Bacc(
        "TRN2", target_bir_lowering=False, debug=False, num_devices=NCORES
    )

    # DRAM I/O (host pre-transposes and pre-rounds to the f32r grid; qT
    # carries an extra all-ones row so bq rides in Wq's last row).
    # qT padded to 128 rows: partition counts that aren't multiples of 16
    # make the DMA pathologically slow (~40us for 0.8MB measured at 101
    # rows vs ~3us at 128).
    qT = nc.dram_tensor("qT", [P, T], f32r, kind="ExternalInput").ap()
    # kvT[p, 4096*pc + u]: u<2048 -> kt piece pc (cols 512c+j), u>=2048 ->
    # vt piece pc.  Host pre-arranges so every piece is ONE fully
    # contiguous transfer (contiguous DMA measured 425GB/s vs 187GB/s for
    # the strided multi-chunk AP; single-ring FIFO avoids the ~4x loss
    # from concurrent-ring thrash).
    kvT = nc.dram_tensor("kvT", [P, 8 * T], f32r, kind="ExternalInput").ap()
    wpk = nc.dram_tensor("wpk", [P, WPACK_W], f32r, kind="ExternalInput").ap()
    wpx = nc.dram_tensor("wpx", [P, WPACKX_W], f32, kind="ExternalInput").ap()
    y = nc.dram_tensor("y", [HPC * 256, D], u8, kind="ExternalOutput").ap()

    with tile.TileContext(nc) as tc, ExitStack() as ctx:
        pool = lambda name, bufs, space="SBUF": ctx.enter_context(
            tc.tile_pool(name=name, bufs=bufs, space=space)
        )
        persist = pool("persist", 1)      # distinct tags -> own slots
        s_pool = pool("s_pool", 3)        # masked S tiles (fp16)
        t_pool = pool("t_pool", 4)        # spike-chain temporaries
        m_pool = pool("m_pool", 2)        # M snapshots (block-diag masked)
        y_pool = pool("y_pool", 3)        # output staging
        pp = pool("pp", 2, "PSUM")        # projections + final (fp32)
        ps = pool("ps", 1, "PSUM")        # S tiles (fp16, 2 parity tags)
        po = pool("po", 2, "PSUM")        # O accumulators (fp16, pair tiles)
        pm = pool("pm", 1, "PSUM")        # persistent M state (fp16)
        pt_ps = pool("pt_ps", 1, "PSUM")  # transpose staging (fp16)

        def ptile(shape, dtype, *, name):
            return persist.tile(shape, dtype, name=name, tag=name)

        # ---- SBUF allocations -----------------------------------------
        qt_sb = ptile([P, T], f32r, name="qt_sb")
        # per-piece k/v tiles: one contiguous [128, 4096] tile per piece
        # (kt cols 0:2048 = 512*c+j, vt cols 2048:4096).
        PW = T // NPIECE
        kv_p = [ptile([P, 2 * KC * PW], f32r, name=f"kv_p{pc}") for pc in range(NPIECE)]
        kt_p = [kv_p[pc][:, 0 : KC * PW] for pc in range(NPIECE)]
        vt_p = [kv_p[pc][:, KC * PW : 2 * KC * PW] for pc in range(NPIECE)]
        wp_sb = ptile([P, WPACK_W], f32r, name="wp_sb")
        wx_sb = ptile([P, WPACKX_W], f32, name="wx_sb")
        wk_sb = [wp_sb[:, OFF_WK + 256 * c :][:, 0:DH] for c in range(KC)]
        wv_sb = [wp_sb[:, OFF_WV + 256 * c :][:, 0:DH] for c in range(KC)]
        wq_sb = wp_sb[:, OFF_WQ : OFF_WQ + DH]
        wo_sb = [
            wx_sb[:, XOFF_WO + 256 * c : XOFF_WO + 256 * (c + 1)].bitcast(f16)
            for c in range(KC)
        ]
        msk_sb = wx_sb[:, XOFF_MSK : XOFF_MSK + 2 * P]               # [128,256]
        mdg_sb = wx_sb[:, XOFF_MDG : XOFF_MDG + P]                   # [128,128]
        bias_sb = wp_sb[:, OFF_BIAS:WPACK_W]
        bo_sb = wx_sb[:, XOFF_BO : XOFF_BO + 2 * P].bitcast(f16)     # [128,512]
        ones_sb = ptile([1, D], f32r, name="ones_sb")
        idt_sb = ptile([P, P], f16, name="idt_sb")
        idt_ones = ptile([1, P], f16, name="idt_ones")
        if has_bo:
            nc.vector.memset(idt_ones[:, :], 1.0)
        # qs/ks: spiked projections, d-major [dk, T]; tile i holds heads
        # 2i (parts 0:64) and 2i+1 (parts 64:128).
        qs = [ptile([P, T], f16, name=f"qs{i}") for i in range(2)]
        ks = [ptile([P, T], f16, name=f"ks{i}") for i in range(2)]
        # vkn: t-major spiked v for all 4 heads (cols 256t + 64*hl), fp16.
        vkn = ptile([P, DH * NT], f16, name="vkn")
        # kn: t-major spiked k, pair-major: cols 256t + 128*pair + 64*(hl%2)
        kn = ptile([P, DH * NT], f16, name="kn")
        # xs: spiked attention output, xs[p, 256*tt + 128*i + 64*sub + d]
        # (head h = 2i+sub): both the per-block spike write and the final
        # projection lhsT are fully contiguous [128,128] views.
        xs = ptile([P, 1024 * HPC], f16, name="xs")

        # ---- loads ----------------------------------------------------
        # The DMA ring fair-shares bandwidth across in-flight transfers;
        # tiny gate-copies (read prev dest, write next dest) order them.
        # Two parallel HWDGE FIFO rings: sync (qSPDynamicHW) carries wpk,
        # qt, vt and wpx; scalar (qActDynamicHW) carries kt.  Each piece's
        # transfers are issued JUST BEFORE the compute that consumes the
        # previous piece, so completion-semaphore waits never cover
        # later-issued transfers.
        def load_piece(pc):
            nc.sync.dma_start(
                out=kv_p[pc][:, :], in_=kvT[:, ts(pc, 2 * KC * PW)]
            )

        # Single sync ring, FIFO, priority order; all transfers contiguous.
        nc.sync.dma_start(out=wp_sb[:, :], in_=wpk[:, :])
        nc.sync.dma_start(out=qt_sb[:, :], in_=qT[:, :])
        load_piece(0)
        nc.sync.dma_start(out=wx_sb[:, :], in_=wpx[:, :])
        nc.vector.memset(ones_sb[:, :].bitcast(f32), 1.0)
        masks.make_identity(nc, idt_sb[:, :])

        BIG = float(2 ** 26)

        def spike_chain(out_ap, in_ap, nm):
            """spike(x) via two exact Relu ops on the ACT engine."""
            tmp = t_pool.tile([P, in_ap.free_size()],
                              f32, name=f"tmp_{nm}", tag=f"tmp_{nm}")
            nc.scalar.activation(tmp[:, :], in_ap, AF.Relu, bias=1.0, scale=-1.0)
            nc.scalar.activation(out_ap, tmp[:, :], AF.Relu, bias=1.0, scale=-BIG)

        # ---- per-piece projections ------------------------------------
        def q_piece(pc):
            for half in range(2):
                pt = pp.tile([P, 512], f32, name="pt", tag="pt")
                nc.tensor.matmul(
                    pt[:, :],
                    lhsT=wq_sb[: DIN + 1, ts(half, P)],
                    rhs=qt_sb[: DIN + 1, ts(pc, 512)],
                    start=True,
                    stop=True,
                )
                spike_chain(qs[half][:, ts(pc, 512)], pt[:, :], "q")

        def ks_chunk(ch):
            for half in range(2):
                pt = pp.tile([P, 512], f32, name="pt", tag="pt")
                for c in range(KC):
                    nc.tensor.matmul(
                        pt[:, :],
                        lhsT=wk_sb[c][:, ts(half, P)],
                        rhs=kt_p[ch][:, ts(c, PW)],
                        start=(c == 0),
                        stop=(c == KC - 1) and not has_bk,
                    )
                if has_bk:
                    nc.tensor.matmul(
                        pt[:, :],
                        lhsT=bias_sb[0:1, ts(half, P)],
                        rhs=ones_sb[0:1, 0:512],
                        start=False,
                        stop=True,
                    )
                spike_chain(ks[half][:, ts(ch, 512)], pt[:, :], "k")
            # t-major spiked K via PE transpose; both pair tiles land in one
            # PSUM tile so a single DVE copy moves them (kn block is
            # contiguous in the pair-major layout the M-update wants).
            for tt in range(4 * ch, 4 * ch + 4):
                tp = pt_ps.tile([P, 2 * P], f16, name="tp", tag="tp")
                for pr in range(2):
                    nc.tensor.transpose(
                        tp[:, ts(pr, P)], ks[pr][:, ts(tt, P)], idt_sb[:, :]
                    )
                nc.vector.tensor_copy(kn[:, ts(tt, DH)], tp[:, :])

        def vkn_block(tt):
            pt = pp.tile([P, 512], f32, name="pt", tag="pt")
            for c in range(KC):
                nc.tensor.matmul(
                    pt[:, 0:DH],
                    lhsT=vt_p[tt // 4][:, PW * c + P * (tt % 4) :][:, 0:P],
                    rhs=wv_sb[c][:, :],
                    start=(c == 0),
                    stop=(c == KC - 1) and not has_bv,
                )
            if has_bv:
                nc.tensor.matmul(
                    pt[:, 0:DH],
                    lhsT=ones_sb[0:1, 0:P],
                    rhs=bias_sb[1:2, 0:DH],
                    start=False,
                    stop=True,
                )
            spike_chain(vkn[:, ts(tt, DH)], pt[:, 0:DH], "v")

        # ---- attention ------------------------------------------------
        pm_t = pm.tile([P, DH], f32, name="pm_t")

        def attn_block(tt):
            if tt > 0:
                # snapshot M_(<tt), block-diag masked so the merged
                # per-pair O_inter matmul sees zero cross-head terms.
                m_sb = [
                    m_pool.tile([P, P], f16, name=f"m_sb{i}", tag=f"m_sb{i}")
                    for i in range(2)
                ]
                for i in range(2):
                    nc.vector.tensor_tensor(
                        m_sb[i][:, :], pm_t[:, ts(i, P)], mdg_sb[:, :],
                        op=ALU.mult,
                    )
            else:
                m_sb = None
            s_ps = [
                ps.tile([P, DH], f32, name=f"s_ps{par}", tag=f"s_ps{par}")
                for par in range(2)
            ]
            for hl in range(HPC):
                par, idx = hl % 2, hl // 2
                rows = slice(64 * par, 64 * par + 64)
                nc.tensor.matmul(
                    s_ps[par][:, ts(idx, P)],
                    lhsT=ks[idx][rows, ts(tt, P)],
                    rhs=qs[idx][rows, ts(tt, P)],
                    start=True,
                    stop=True,
                )
            s_sb = [
                s_pool.tile([P, DH], f16, name=f"s_sb{par}", tag=f"s_sb{par}")
                for par in range(2)
            ]
            for par in range(2):
                nc.vector.tensor_tensor(
                    s_sb[par][:, :], s_ps[par][:, :], msk_sb[:, :], op=ALU.mult
                )
            # o_ps[i]: [tq, dv of heads 2i (cols 0:64), 2i+1 (64:128)]
            # The two intra matmuls share ONE start: start=True clears the
            # has_written bits of the whole PSUM zero region, so a second
            # start would make the full-width O_inter OVERWRITE (not
            # accumulate onto) the first head's columns.
            o_ps = [po.tile([P, P], f32, name="o_ps") for _ in range(2)]
            for i in range(2):
                for par in range(2):
                    hl = 2 * i + par
                    nc.tensor.matmul(
                        o_ps[i][:, ts(par, 64)],
                        lhsT=s_sb[par][:, ts(i, P)],
                        rhs=vkn[:, DH * tt + 64 * hl :][:, 0:64],
                        start=(par == 0),
                        stop=(tt == 0),
                        skip_group_check=True,
                    )
            if tt > 0:
                for i in range(2):
                    nc.tensor.matmul(
                        o_ps[i][:, :],
                        lhsT=qs[i][:, ts(tt, P)],
                        rhs=m_sb[i][:, :],
                        start=False,
                        stop=True,
                        skip_group_check=True,
                    )
            # M += K_pair^T V_pair: one K=128,N=128 matmul per head pair;
            # only the diagonal 64x64 blocks are meaningful (snapshot is
            # masked).  stop=True closes the group so the snapshot read is
            # legal; on HW stop is a no-op and start=False keeps summing.
            for pr in range(2):
                nc.tensor.matmul(
                    pm_t[:, ts(pr, P)],
                    lhsT=kn[:, DH * tt + P * pr :][:, 0:P],
                    rhs=vkn[:, DH * tt + P * pr :][:, 0:P],
                    start=(tt == 0 and pr == 0),
                    stop=(pr == 1),
                    skip_group_check=True,
                )
            # x = spike(scale * O) = (O >= 1/scale): one DVE op per pair.
            for i in range(2):
                nc.vector.tensor_scalar(
                    xs[:, DH * tt + P * i :][:, 0:P],
                    o_ps[i][:, :],
                    inv_scale,
                    None,
                    ALU.is_ge,
                )

        # ---- final projection (fp16: xs {0,1} x Wo fp16) --------------
        # Output rows r with r%4 == m contract only over attention piece m
        # (X[r, f] = x_att[t=512*(r%4)+f, d=r//4]).  With the xs layout the
        # lhsT for (piece m, chunk cc, pair j) is the contiguous block
        # xs[:, 256*(4m+cc) + 128j :][:128].

        def final_piece(m):
            for j in range(2):  # head pair: heads 2j, 2j+1
                yp = pp.tile([P, 512], f32, name="pt", tag="pt")
                for cc in range(KC):
                    nc.tensor.matmul(
                        yp[:, :],
                        lhsT=xs[:, DH * (4 * m + cc) + P * j :][:, 0:P],
                        rhs=wo_sb[cc][:, :],
                        start=(cc == 0),
                        stop=(cc == KC - 1) and not has_bo,
                    )
                if has_bo:
                    nc.tensor.matmul(
                        yp[:, :],
                        lhsT=idt_ones[0:1, 0:P],
                        rhs=bo_sb[0:1, 0:512],
                        start=False,
                        stop=True,
                    )
                y_sb = y_pool.tile([P, D], u8, name="y_sb")
                nc.vector.tensor_scalar(
                    y_sb[:, :], yp[:, :], 1.0, None, ALU.is_ge
                )
                for sub in range(2):
                    h = 2 * j + sub
                    nc.gpsimd.dma_start(
                        out=y[256 * h + m : 256 * (h + 1) : 4, :],
                        in_=y_sb[64 * sub : 64 * sub + 64, :],
                    )

        # ---- schedule -------------------------------------------------
        def proj_piece(pc):
            q_piece(pc)
            ks_chunk(pc)
            for tt in range(4 * pc, 4 * pc + 4):
                vkn_block(tt)

        load_piece(1)
        proj_piece(0)
        load_piece(2)
        proj_piece(1)
        for pc in range(4):
            if pc + 2 < 4:
                if pc + 3 < 4:
                    load_piece(pc + 3)
                proj_piece(pc + 2)
            for tt in range(4 * pc, 4 * pc + 4):
                attn_block(tt)
            final_piece(pc)

    nc.compile()
    return nc


def _get_prog(scale, has_bk, has_bv, has_bo):
    key = (scale, has_bk, has_bv, has_bo)
    if key not in _prog_cache:
        _prog_cache[key] = _build(scale, has_bk, has_bv, has_bo)
    return _prog_cache[key]


def _pack_weights(Wq, bq, Wk, bk, Wv, bv, Wo, bo, cs):
    wpk = np.zeros((P, WPACK_W), np.float32)
    wpx = np.zeros((P, WPACKX_W), np.float32)
    for c in range(KC):
        wpk[:, OFF_WK + 256 * c : OFF_WK + 256 * (c + 1)] = _f32r_round(
            Wk[128 * c : 128 * (c + 1), cs]
        )
        wpk[:, OFF_WV + 256 * c : OFF_WV + 256 * (c + 1)] = _f32r_round(
            Wv[128 * c : 128 * (c + 1), cs]
        )
        # Wo fp16 pairs packed into f32 words
        wo16 = np.ascontiguousarray(
            Wo[128 * c : 128 * (c + 1), :].astype(np.float16)
        )
        wpx[:, XOFF_WO + 256 * c : XOFF_WO + 256 * (c + 1)] = wo16.view(
            np.float32
        )
    wq = np.zeros((P, DH), np.float32)
    wq[:DIN] = Wq[:, cs]
    wq[DIN] = bq[cs]
    wpk[:, OFF_WQ : OFF_WQ + DH] = _f32r_round(wq)
    wpx[:, XOFF_MSK : XOFF_MSK + 2 * P] = np.tile(
        np.triu(np.ones((P, P), np.float32)), (1, 2)
    )
    wpx[:, XOFF_MDG : XOFF_MDG + P] = np.kron(
        np.eye(2, dtype=np.float32), np.ones((64, 64), np.float32)
    )
    wpk[0, OFF_BIAS : OFF_BIAS + DH] = _f32r_round(
        np.ascontiguousarray(bk[cs], np.float32)
    )
    wpk[1, OFF_BIAS : OFF_BIAS + DH] = _f32r_round(
        np.ascontiguousarray(bv[cs], np.float32)
    )
    bo16 = np.ascontiguousarray(bo.astype(np.float16))
    wpx[0, XOFF_BO : XOFF_BO + D // 2] = bo16.view(np.float32)
    return wpk, wpx


def kernel(**inputs) -> np.ndarray:
    global last_exec_time_ns
    from concourse.bass_utils import run_bass_kernel_spmd

    g = lambda n: np.asarray(inputs[n], dtype=np.float32)
    query, key, value = g("query"), g("key"), g("value")
    Wq, bq, Wk, bk = g("Wq"), g("bq"), g("Wk"), g("bk")
    Wv, bv, Wo, bo = g("Wv"), g("bv"), g("Wo"), g("bo")
    scale = float(np.asarray(inputs["scale"], dtype=np.float32).reshape(-1)[0])

    has_bk, has_bv, has_bo = (bool(np.any(x)) for x in (bk, bv, bo))
    prog = _get_prog(scale, has_bk, has_bv, has_bo)

    # pre-round the shared per-batch data once; arrange kt/vt piece-major
    # so each piece is one contiguous DMA:
    #   kvT[p, 4096*pc + 512*c + j]        = kT[128c+p, 512pc+j]
    #   kvT[p, 4096*pc + 2048 + 512*c + j] = vT[128c+p, 512pc+j]
    qTr, kvTr = [None] * B, [None] * B
    for b in range(B):
        qTa = np.zeros((P, T), np.float32)
        qTa[:DIN] = _f32r_round(np.ascontiguousarray(query[b].T))
        qTa[DIN] = 1.0
        qTr[b] = qTa
        kk = _f32r_round(np.ascontiguousarray(key[b].T))
        vv = _f32r_round(np.ascontiguousarray(value[b].T))
        # [512, 2048] -> [pc, p, 512c+j]
        ka = kk.reshape(KC, P, NPIECE, 512).transpose(2, 1, 0, 3).reshape(
            NPIECE, P, KC * 512
        )
        va = vv.reshape(KC, P, NPIECE, 512).transpose(2, 1, 0, 3).reshape(
            NPIECE, P, KC * 512
        )
        kv = np.concatenate([ka, va], axis=2)  # [pc, p, 4096]
        kvTr[b] = np.ascontiguousarray(kv.transpose(1, 0, 2)).reshape(P, 8 * T)

    in_maps = []
    for c in range(NCORES):
        b, hg = divmod(c, 2)
        cs = slice(DH * hg, DH * (hg + 1))
        wpk_c, wpx_c = _pack_weights(Wq, bq, Wk, bk, Wv, bv, Wo, bo, cs)
        in_maps.append(
            {
                "qT": qTr[b],
                "kvT": kvTr[b],
                "wpk": wpk_c,
                "wpx": wpx_c,
            }
        )

    trace = os.environ.get("BASS_TRACE", "") not in ("", "0")
    res = run_bass_kernel_spmd(
        prog, in_maps, core_ids=list(range(NCORES)), trace=trace
    )
    last_exec_time_ns = res.exec_time_ns
    if res.exec_time_ns is not None:
        print(f"HW exec time: {res.exec_time_ns} ns")

    out = np.empty((B, T, D), np.float32)
    for c in range(NCORES):
        b, hg = divmod(c, 2)
        out[b, 1024 * hg : 1024 * (hg + 1)] = res.results[c]["y"].astype(
            np.float32
        )
    return out


# revision 22
# speedup vs baseline: 1.5979x; 1.0486x over previous
"""Trainium2 Bass kernel: spiking multi-head attention (nn_MultiHeadedAttention).

Reference semantics (B=4, T=2048, DIN=100, D=512, h=8 heads, dk=64):
    q = spike(query @ Wq + bq)   (spike = (x >= 1.0) -> {0,1})
    k = spike(key @ Wk + bk);  v = spike(value @ Wv + bv)
    attn = (q @ k^T) * scale, causally masked (keep k<=q), NO softmax
    x = spike(attn @ v)
    x = x.transpose(0,1,3,2).reshape(B,T,h*dk)    # scrambled reshape
    y = spike(x @ Wo + bo)

Key facts exploited:
  * No softmax -> causal attention is LINEAR attention:
        O_t = q_t . M_t  +  intra-block tril(Q K^T) V,   M = sum_j k_j v_j^T
    so only the 16 diagonal 128x128 S-tiles per head are materialized.
  * The scrambled reshape maps output rows [256*h, 256*(h+1)) to exactly one
    head h, so head-parallel sharding needs NO cross-core communication.
  * float32r matmuls stream 1 col/cycle for N>=256 (4x fp32, measured 227ns
    vs 860ns at N=512) with fp32-range 12-bit-mantissa operands; the host
    pre-rounds all real-valued operands to the f32r grid so every projection
    runs at bf16 speed.  End-to-end spike-flip error from the rounding is
    ~1.3e-2 rel (gate 2e-2), verified against the reference on CPU.
  * Spiked tensors are {0,1} and S / O / M are integers, all exact in fp16;
    the attention core uses fp16 operands AND fp16 PSUM tiles (integer
    values -> exact; halves DVE PSUM-read cost and PSUM bank usage).
  * O_inter is ONE matmul per head pair (lhsT = full 128-row qs tile times a
    block-diag-masked M snapshot), halving its LDWEIGHTS cost.
  * Final projection: xs {0,1} fp16 x Wo fp16 (adds ~51 spike flips), y
    spike emitted as uint8 (host upcasts) cutting output DMA 4x.

Sharding: core c -> batch b=c//2, head-group hg=c%2 (4 heads per core).

Hardware notes encoded below:
  * K=64 matmuls whose lhsT sits at partition base 0 vs base 64 execute
    concurrently in disjoint PE row groups; concurrent writes to one PSUM
    bank hang the device -> parity-0/1 S tiles live in different banks
    (PSUM pool slots are bank-padded, one slot per tag).
  * PSUM budget is exactly 8 banks: pp(2, fp32 projections+final) ps(2,
    fp16 S parities) po(2, fp16 O pairs) pm(1, fp16 M state) tp(1, fp16
    transposes).
  * Engine balance: DVE does the tensor_tensor masks + y/kn; ACT+GPSIMD
    run the two-op spike chains (GPSIMD cannot read PSUM, so it always
    takes the SBUF-side second op).
  * DMA-issue instructions are expensive on the issuing engine; all weights
    ride one packed f32r tensor and k/v loads are paced with gate-copies so
    early-needed pieces get full ring bandwidth.
"""

import os
import numpy as np

B, T, DIN, D = 4, 2048, 100, 512
H, DK = 8, 64
NCORES = 8
HPC = 4          # heads per core
DH = HPC * DK    # 256 projected features per core
P = 128
NT = T // P      # 16 t-blocks
KC = D // P      # 4 contraction chunks of the D=512 dim
NPIECE = 4       # pieces along T (512 each)

# packed weights ride in two tensors: wpk (f32r: everything consumed by
# f32r matmuls -- the compiler re-rounds f32r DRAM data, so bit-packed f16
# payloads must NOT live there) and wpx (plain f32: Wo f16 pairs + masks).
OFF_WK = 0                      # 4 chunks x 256 = 1024 f32r cols
OFF_WV = 1024                   # 1024 cols
OFF_WQ = 2048                   # 256 cols
OFF_BIAS = 2304                 # rows 0/1 = bk/bv (f32r)
WPACK_W = 2560                  # f32r tensor width
XOFF_WO = 0                     # Wo fp16 pairs: 4 chunks x 512 f16 = 1024 f32 cols
XOFF_MSK = 1024                 # triu mask f32: 256 cols
XOFF_MDG = 1280                 # block-diag mask f32: 128 cols
XOFF_BO = 1408                  # row 0 = bo (fp16 pairs, 256 f32 cols)
WPACKX_W = 1664                 # f32 tensor width

_prog_cache: dict = {}
last_exec_time_ns = None


def _f32r_round(x: np.ndarray) -> np.ndarray:
    """Round fp32 to the f32r grid (11 explicit mantissa bits, RN)."""
    u = np.ascontiguousarray(x, np.float32).view(np.uint32)
    r = (u + np.uint32(0x7FF) + ((u >> np.uint32(12)) & np.uint32(1))) & np.uint32(
        0xFFFFF000
    )
    return r.view(np.float32)


def _build(scale: float, has_bk: bool, has_bv: bool, has_bo: bool):
    from contextlib import ExitStack

    import concourse.bass as bass
    import concourse.tile as tile
    import concourse.mybir as mybir
    from concourse import bacc
    from concourse.bass import ts
    from concourse import masks

    f32 = mybir.dt.float32
    f32r = mybir.dt.float32r
    f16 = mybir.dt.float16
    u8 = mybir.dt.uint8
    ALU = mybir.AluOpType
    AF = mybir.ActivationFunctionType
    inv_scale = 1.0 / scale

    nc = bacc.Bacc(
        "TRN2", target_bir_lowering=False, debug=False, num_devices=NCORES
    )

    # DRAM I/O (host pre-transposes and pre-rounds to the f32r grid; qT
    # carries an extra all-ones row so bq rides in Wq's last row).
    # qT padded to 128 rows: partition counts that aren't multiples of 16
    # make the DMA pathologically slow (~40us for 0.8MB measured at 101
    # rows vs ~3us at 128).
    qT = nc.dram_tensor("qT", [P, T], f32r, kind="ExternalInput").ap()
    # kvT[p, 4096*pc + u]: u<2048 -> kt piece pc (cols 512c+j), u>=2048 ->
    # vt piece pc.  Host pre-arranges so every piece is ONE fully
    # contiguous transfer (contiguous DMA measured 425GB/s vs 187GB/s for
    # the strided multi-chunk AP; single-ring FIFO avoids the ~4x loss
    # from concurrent-ring thrash).
    kvT = nc.dram_tensor("kvT", [P, 8 * T], f32r, kind="ExternalInput").ap()
    wpk = nc.dram_tensor("wpk", [P, WPACK_W], f32r, kind="ExternalInput").ap()
    wpx = nc.dram_tensor("wpx", [P, WPACKX_W], f32, kind="ExternalInput").ap()
    y = nc.dram_tensor("y", [HPC * 256, D], u8, kind="ExternalOutput").ap()

    with tile.TileContext(nc) as tc, ExitStack() as ctx:
        pool = lambda name, bufs, space="SBUF": ctx.enter_context(
            tc.tile_pool(name=name, bufs=bufs, space=space)
        )
        persist = pool("persist", 1)      # distinct tags -> own slots
        s_pool = pool("s_pool", 3)        # masked S tiles (fp16)
        t_pool = pool("t_pool", 4)        # spike-chain temporaries
        m_pool = pool("m_pool", 2)        # M snapshots (block-diag masked)
        y_pool = pool("y_pool", 3)        # output staging
        pp = pool("pp", 2, "PSUM")        # projections + final (fp32)
        ps = pool("ps", 1, "PSUM")        # S tiles (fp16, 2 parity tags)
        po = pool("po", 2, "PSUM")        # O accumulators (fp16, pair tiles)
        pm = pool("pm", 1, "PSUM")        # persistent M state (fp16)
        pt_ps = pool("pt_ps", 1, "PSUM")  # transpose staging (fp16)

        def ptile(shape, dtype, *, name):
            return persist.tile(shape, dtype, name=name, tag=name)

        # ---- SBUF allocations -----------------------------------------
        qt_sb = ptile([P, T], f32r, name="qt_sb")
        # per-piece k/v tiles: one contiguous [128, 4096] tile per piece
        # (kt cols 0:2048 = 512*c+j, vt cols 2048:4096).
        PW = T // NPIECE
        kv_p = [ptile([P, 2 * KC * PW], f32r, name=f"kv_p{pc}") for pc in range(NPIECE)]
        kt_p = [kv_p[pc][:, 0 : KC * PW] for pc in range(NPIECE)]
        vt_p = [kv_p[pc][:, KC * PW : 2 * KC * PW] for pc in range(NPIECE)]
        wp_sb = ptile([P, WPACK_W], f32r, name="wp_sb")
        wx_sb = ptile([P, WPACKX_W], f32, name="wx_sb")
        wk_sb = [wp_sb[:, OFF_WK + 256 * c :][:, 0:DH] for c in range(KC)]
        wv_sb = [wp_sb[:, OFF_WV + 256 * c :][:, 0:DH] for c in range(KC)]
        wq_sb = wp_sb[:, OFF_WQ : OFF_WQ + DH]
        wo_sb = [
            wx_sb[:, XOFF_WO + 256 * c : XOFF_WO + 256 * (c + 1)].bitcast(f16)
            for c in range(KC)
        ]
        msk_sb = wx_sb[:, XOFF_MSK : XOFF_MSK + 2 * P]               # [128,256]
        mdg_sb = wx_sb[:, XOFF_MDG : XOFF_MDG + P]                   # [128,128]
        bias_sb = wp_sb[:, OFF_BIAS:WPACK_W]
        bo_sb = wx_sb[:, XOFF_BO : XOFF_BO + 2 * P].bitcast(f16)     # [128,512]
        ones_sb = ptile([1, D], f32r, name="ones_sb")
        idt_sb = ptile([P, P], f16, name="idt_sb")
        idt_ones = ptile([1, P], f16, name="idt_ones")
        if has_bo:
            nc.vector.memset(idt_ones[:, :], 1.0)
        # qs/ks: spiked projections, d-major [dk, T]; tile i holds heads
        # 2i (parts 0:64) and 2i+1 (parts 64:128).
        qs = [ptile([P, T], f16, name=f"qs{i}") for i in range(2)]
        ks = [ptile([P, T], f16, name=f"ks{i}") for i in range(2)]
        # vkn: t-major spiked v for all 4 heads (cols 256t + 64*hl), fp16.
        vkn = ptile([P, DH * NT], f16, name="vkn")
        # kn: t-major spiked k, pair-major: cols 256t + 128*pair + 64*(hl%2)
        kn = ptile([P, DH * NT], f16, name="kn")
        # xs: spiked attention output, xs[p, 256*tt + 128*i + 64*sub + d]
        # (head h = 2i+sub): both the per-block spike write and the final
        # projection lhsT are fully contiguous [128,128] views.
        xs = ptile([P, 1024 * HPC], f16, name="xs")

        # ---- loads ----------------------------------------------------
        # The DMA ring fair-shares bandwidth across in-flight transfers;
        # tiny gate-copies (read prev dest, write next dest) order them.
        # Two parallel HWDGE FIFO rings: sync (qSPDynamicHW) carries wpk,
        # qt, vt and wpx; scalar (qActDynamicHW) carries kt.  Each piece's
        # transfers are issued JUST BEFORE the compute that consumes the
        # previous piece, so completion-semaphore waits never cover
        # later-issued transfers.
        def load_piece(pc):
            nc.sync.dma_start(
                out=kv_p[pc][:, :], in_=kvT[:, ts(pc, 2 * KC * PW)]
            )

        # Single sync ring, FIFO, priority order; all transfers contiguous.
        nc.sync.dma_start(out=wp_sb[:, :], in_=wpk[:, :])
        nc.sync.dma_start(out=qt_sb[:, :], in_=qT[:, :])
        load_piece(0)
        nc.sync.dma_start(out=wx_sb[:, :], in_=wpx[:, :])
        nc.vector.memset(ones_sb[:, :].bitcast(f32), 1.0)
        masks.make_identity(nc, idt_sb[:, :])

        BIG = float(2 ** 26)

        def spike_chain(out_ap, in_ap, nm):
            """spike(x) via two exact Relu ops on the ACT engine."""
            tmp = t_pool.tile([P, in_ap.free_size()],
                              f32, name=f"tmp_{nm}", tag=f"tmp_{nm}")
            nc.scalar.activation(tmp[:, :], in_ap, AF.Relu, bias=1.0, scale=-1.0)
            nc.scalar.activation(out_ap, tmp[:, :], AF.Relu, bias=1.0, scale=-BIG)

        # ---- per-piece projections ------------------------------------
        def q_piece(pc):
            for half in range(2):
                pt = pp.tile([P, 512], f32, name="pt", tag="pt")
                nc.tensor.matmul(
                    pt[:, :],
                    lhsT=wq_sb[: DIN + 1, ts(half, P)],
                    rhs=qt_sb[: DIN + 1, ts(pc, 512)],
                    start=True,
                    stop=True,
                )
                spike_chain(qs[half][:, ts(pc, 512)], pt[:, :], "q")

        def ks_chunk(ch):
            for half in range(2):
                pt = pp.tile([P, 512], f32, name="pt", tag="pt")
                for c in range(KC):
                    nc.tensor.matmul(
                        pt[:, :],
                        lhsT=wk_sb[c][:, ts(half, P)],
                        rhs=kt_p[ch][:, ts(c, PW)],
                        start=(c == 0),
                        stop=(c == KC - 1) and not has_bk,
                    )
                if has_bk:
                    nc.tensor.matmul(
                        pt[:, :],
                        lhsT=bias_sb[0:1, ts(half, P)],
                        rhs=ones_sb[0:1, 0:512],
                        start=False,
                        stop=True,
                    )
                spike_chain(ks[half][:, ts(ch, 512)], pt[:, :], "k")
            # t-major spiked K via PE transpose; both pair tiles land in one
            # PSUM tile so a single DVE copy moves them (kn block is
            # contiguous in the pair-major layout the M-update wants).
            for tt in range(4 * ch, 4 * ch + 4):
                tp = pt_ps.tile([P, 2 * P], f16, name="tp", tag="tp")
                for pr in range(2):
                    nc.tensor.transpose(
                        tp[:, ts(pr, P)], ks[pr][:, ts(tt, P)], idt_sb[:, :]
                    )
                nc.vector.tensor_copy(kn[:, ts(tt, DH)], tp[:, :])

        def vkn_block(tt):
            pt = pp.tile([P, 512], f32, name="pt", tag="pt")
            for c in range(KC):
                nc.tensor.matmul(
                    pt[:, 0:DH],
                    lhsT=vt_p[tt // 4][:, PW * c + P * (tt % 4) :][:, 0:P],
                    rhs=wv_sb[c][:, :],
                    start=(c == 0),
                    stop=(c == KC - 1) and not has_bv,
                )
            if has_bv:
                nc.tensor.matmul(
                    pt[:, 0:DH],
                    lhsT=ones_sb[0:1, 0:P],
                    rhs=bias_sb[1:2, 0:DH],
                    start=False,
                    stop=True,
                )
            spike_chain(vkn[:, ts(tt, DH)], pt[:, 0:DH], "v")

        # ---- attention ------------------------------------------------
        pm_t = pm.tile([P, DH], f32, name="pm_t")

        def attn_block(tt):
            if tt > 0:
                # snapshot M_(<tt), block-diag masked so the merged
                # per-pair O_inter matmul sees zero cross-head terms.
                m_sb = [
                    m_pool.tile([P, P], f16, name=f"m_sb{i}", tag=f"m_sb{i}")
                    for i in range(2)
                ]
                for i in range(2):
                    nc.vector.tensor_tensor(
                        m_sb[i][:, :], pm_t[:, ts(i, P)], mdg_sb[:, :],
                        op=ALU.mult,
                    )
            else:
                m_sb = None
            s_ps = [
                ps.tile([P, DH], f32, name=f"s_ps{par}", tag=f"s_ps{par}")
                for par in range(2)
            ]
            for hl in range(HPC):
                par, idx = hl % 2, hl // 2
                rows = slice(64 * par, 64 * par + 64)
                nc.tensor.matmul(
                    s_ps[par][:, ts(idx, P)],
                    lhsT=ks[idx][rows, ts(tt, P)],
                    rhs=qs[idx][rows, ts(tt, P)],
                    start=True,
                    stop=True,
                )
            s_sb = [
                s_pool.tile([P, DH], f16, name=f"s_sb{par}", tag=f"s_sb{par}")
                for par in range(2)
            ]
            for par in range(2):
                nc.vector.tensor_tensor(
                    s_sb[par][:, :], s_ps[par][:, :], msk_sb[:, :], op=ALU.mult
                )
            # o_ps: [tq, 128i + 64par + dv] for head 2i+par -- one tile,
            # matching the xs layout so the x spike is ONE contiguous DVE
            # op.  Each 512B PSUM zero region (128-col half) shares ONE
            # start: a second start would clear has_written and make the
            # O_inter accumulate overwrite the first head's columns.
            o_ps = po.tile([P, 2 * P], f32, name="o_ps")
            for i in range(2):
                for par in range(2):
                    hl = 2 * i + par
                    nc.tensor.matmul(
                        o_ps[:, P * i + 64 * par :][:, 0:64],
                        lhsT=s_sb[par][:, ts(i, P)],
                        rhs=vkn[:, DH * tt + 64 * hl :][:, 0:64],
                        start=(i == 0 and par == 0),
                        stop=(tt == 0),
                        skip_group_check=True,
                    )
            if tt > 0:
                for i in range(2):
                    nc.tensor.matmul(
                        o_ps[:, ts(i, P)],
                        lhsT=qs[i][:, ts(tt, P)],
                        rhs=m_sb[i][:, :],
                        start=False,
                        stop=True,
                        skip_group_check=True,
                    )
            # M += K_pair^T V_pair: one K=128,N=128 matmul per head pair;
            # only the diagonal 64x64 blocks are meaningful (snapshot is
            # masked).  stop=True closes the group so the snapshot read is
            # legal; on HW stop is a no-op and start=False keeps summing.
            for pr in range(2):
                nc.tensor.matmul(
                    pm_t[:, ts(pr, P)],
                    lhsT=kn[:, DH * tt + P * pr :][:, 0:P],
                    rhs=vkn[:, DH * tt + P * pr :][:, 0:P],
                    start=(tt == 0 and pr == 0),
                    stop=(pr == 1),
                    skip_group_check=True,
                )
            # x = spike(scale * O) = (O >= 1/scale): ONE DVE op per block.
            nc.vector.tensor_scalar(
                xs[:, ts(tt, DH)], o_ps[:, :], inv_scale, None, ALU.is_ge
            )

        # ---- final projection (fp16: xs {0,1} x Wo fp16) --------------
        # Output rows r with r%4 == m contract only over attention piece m
        # (X[r, f] = x_att[t=512*(r%4)+f, d=r//4]).  With the xs layout the
        # lhsT for (piece m, chunk cc, pair j) is the contiguous block
        # xs[:, 256*(4m+cc) + 128j :][:128].

        def final_piece(m):
            for j in range(2):  # head pair: heads 2j, 2j+1
                yp = pp.tile([P, 512], f32, name="pt", tag="pt")
                for cc in range(KC):
                    nc.tensor.matmul(
                        yp[:, :],
                        lhsT=xs[:, DH * (4 * m + cc) + P * j :][:, 0:P],
                        rhs=wo_sb[cc][:, :],
                        start=(cc == 0),
                        stop=(cc == KC - 1) and not has_bo,
                    )
                if has_bo:
                    nc.tensor.matmul(
                        yp[:, :],
                        lhsT=idt_ones[0:1, 0:P],
                        rhs=bo_sb[0:1, 0:512],
                        start=False,
                        stop=True,
                    )
                y_sb = y_pool.tile([P, D], u8, name="y_sb")
                nc.vector.tensor_scalar(
                    y_sb[:, :], yp[:, :], 1.0, None, ALU.is_ge
                )
                for sub in range(2):
                    h = 2 * j + sub
                    nc.gpsimd.dma_start(
                        out=y[256 * h + m : 256 * (h + 1) : 4, :],
                        in_=y_sb[64 * sub : 64 * sub + 64, :],
                    )

        # ---- schedule -------------------------------------------------
        def proj_piece(pc):
            q_piece(pc)
            ks_chunk(pc)
            for tt in range(4 * pc, 4 * pc + 4):
                vkn_block(tt)

        load_piece(1)
        proj_piece(0)
        load_piece(2)
        proj_piece(1)
        for pc in range(4):
            if pc + 2 < 4:
                if pc + 3 < 4:
                    load_piece(pc + 3)
                proj_piece(pc + 2)
            for tt in range(4 * pc, 4 * pc + 4):
                attn_block(tt)
            final_piece(pc)

    nc.compile()
    return nc


def _get_prog(scale, has_bk, has_bv, has_bo):
    key = (scale, has_bk, has_bv, has_bo)
    if key not in _prog_cache:
        _prog_cache[key] = _build(scale, has_bk, has_bv, has_bo)
    return _prog_cache[key]


def _pack_weights(Wq, bq, Wk, bk, Wv, bv, Wo, bo, cs):
    wpk = np.zeros((P, WPACK_W), np.float32)
    wpx = np.zeros((P, WPACKX_W), np.float32)
    for c in range(KC):
        wpk[:, OFF_WK + 256 * c : OFF_WK + 256 * (c + 1)] = _f32r_round(
            Wk[128 * c : 128 * (c + 1), cs]
        )
        wpk[:, OFF_WV + 256 * c : OFF_WV + 256 * (c + 1)] = _f32r_round(
            Wv[128 * c : 128 * (c + 1), cs]
        )
        # Wo fp16 pairs packed into f32 words
        wo16 = np.ascontiguousarray(
            Wo[128 * c : 128 * (c + 1), :].astype(np.float16)
        )
        wpx[:, XOFF_WO + 256 * c : XOFF_WO + 256 * (c + 1)] = wo16.view(
            np.float32
        )
    wq = np.zeros((P, DH), np.float32)
    wq[:DIN] = Wq[:, cs]
    wq[DIN] = bq[cs]
    wpk[:, OFF_WQ : OFF_WQ + DH] = _f32r_round(wq)
    wpx[:, XOFF_MSK : XOFF_MSK + 2 * P] = np.tile(
        np.triu(np.ones((P, P), np.float32)), (1, 2)
    )
    wpx[:, XOFF_MDG : XOFF_MDG + P] = np.kron(
        np.eye(2, dtype=np.float32), np.ones((64, 64), np.float32)
    )
    wpk[0, OFF_BIAS : OFF_BIAS + DH] = _f32r_round(
        np.ascontiguousarray(bk[cs], np.float32)
    )
    wpk[1, OFF_BIAS : OFF_BIAS + DH] = _f32r_round(
        np.ascontiguousarray(bv[cs], np.float32)
    )
    bo16 = np.ascontiguousarray(bo.astype(np.float16))
    wpx[0, XOFF_BO : XOFF_BO + D // 2] = bo16.view(np.float32)
    return wpk, wpx


def kernel(**inputs) -> np.ndarray:
    global last_exec_time_ns
    from concourse.bass_utils import run_bass_kernel_spmd

    g = lambda n: np.asarray(inputs[n], dtype=np.float32)
    query, key, value = g("query"), g("key"), g("value")
    Wq, bq, Wk, bk = g("Wq"), g("bq"), g("Wk"), g("bk")
    Wv, bv, Wo, bo = g("Wv"), g("bv"), g("Wo"), g("bo")
    scale = float(np.asarray(inputs["scale"], dtype=np.float32).reshape(-1)[0])

    has_bk, has_bv, has_bo = (bool(np.any(x)) for x in (bk, bv, bo))
    prog = _get_prog(scale, has_bk, has_bv, has_bo)

    # pre-round the shared per-batch data once; arrange kt/vt piece-major
    # so each piece is one contiguous DMA:
    #   kvT[p, 4096*pc + 512*c + j]        = kT[128c+p, 512pc+j]
    #   kvT[p, 4096*pc + 2048 + 512*c + j] = vT[128c+p, 512pc+j]
    qTr, kvTr = [None] * B, [None] * B
    for b in range(B):
        qTa = np.zeros((P, T), np.float32)
        qTa[:DIN] = _f32r_round(np.ascontiguousarray(query[b].T))
        qTa[DIN] = 1.0
        qTr[b] = qTa
        kk = _f32r_round(np.ascontiguousarray(key[b].T))
        vv = _f32r_round(np.ascontiguousarray(value[b].T))
        # [512, 2048] -> [pc, p, 512c+j]
        ka = kk.reshape(KC, P, NPIECE, 512).transpose(2, 1, 0, 3).reshape(
            NPIECE, P, KC * 512
        )
        va = vv.reshape(KC, P, NPIECE, 512).transpose(2, 1, 0, 3).reshape(
            NPIECE, P, KC * 512
        )
        kv = np.concatenate([ka, va], axis=2)  # [pc, p, 4096]
        kvTr[b] = np.ascontiguousarray(kv.transpose(1, 0, 2)).reshape(P, 8 * T)

    in_maps = []
    for c in range(NCORES):
        b, hg = divmod(c, 2)
        cs = slice(DH * hg, DH * (hg + 1))
        wpk_c, wpx_c = _pack_weights(Wq, bq, Wk, bk, Wv, bv, Wo, bo, cs)
        in_maps.append(
            {
                "qT": qTr[b],
                "kvT": kvTr[b],
                "wpk": wpk_c,
                "wpx": wpx_c,
            }
        )

    trace = os.environ.get("BASS_TRACE", "") not in ("", "0")
    res = run_bass_kernel_spmd(
        prog, in_maps, core_ids=list(range(NCORES)), trace=trace
    )
    last_exec_time_ns = res.exec_time_ns
    if res.exec_time_ns is not None:
        print(f"HW exec time: {res.exec_time_ns} ns")

    out = np.empty((B, T, D), np.float32)
    for c in range(NCORES):
        b, hg = divmod(c, 2)
        out[b, 1024 * hg : 1024 * (hg + 1)] = res.results[c]["y"].astype(
            np.float32
        )
    return out
